# revision 1
# baseline (speedup 1.0000x reference)
"""Fused GPT-2 transformer block on 8 Trainium2 NeuronCores.

Sharding: 8 cores = 4 batches x 2 causal-balanced folds. Core (b, f) owns the 8
interleaved 128-token blocks of parity f of batch b (queries), and receives all
2048 tokens of batch b as context, permuted [other-parity blocks | own blocks].
Causality is enforced by exact 0/1 mask multiplies after exp, so a single SPMD
program serves all cores. No collectives.

Layouts: LN1(x) is PE-transposed to hT [D, tok] (bf16); Q/K are produced in
head-major transposed layout (bf16), V in token-major layout with an appended
ones column (so the P@V matmul also accumulates softmax denominators).
Exp runs on the scalar engine in [128,1024] slabs straight from PSUM to bf16;
causal masking is a 0/1 elementwise multiply on the vector engine afterwards
(exp(s+m) == exp(s)*exp(m) with exp(m) in {0,1} exactly). proj/fc matmuls
contract against feature-major lhsT slices. All weights travel as bf16;
LN affines and the proj bias are folded on the host.
"""

import contextlib
import os

import numpy as np
import ml_dtypes

import concourse.bass as bass
import concourse.mybir as mybir
import concourse.tile as tile
from concourse import bacc
from concourse.bass_utils import run_bass_kernel_spmd
from concourse.masks import make_identity

F32 = mybir.dt.float32
F32R = mybir.dt.float32r
BF16 = mybir.dt.bfloat16
AF = mybir.ActivationFunctionType
ALU = mybir.AluOpType

B, S, D, H = 4, 2048, 1024, 16
HD = D // H          # 64
DFF = 4 * D          # 4096
EPS = 1e-5
MASKED_BIAS = -10000.0
N_CORES = 8

SB = S // 128        # 16 ctx blocks
OWN = S // 2         # 1024 own tokens
OB = OWN // 128      # 8 own blocks
NQG = 4              # q-groups of 256
QG = 256
HSETS = 4            # head sets
HPS = H // HSETS     # 4 heads per set


def _klist(g):
    """ctx k-block indices computed for q-group g (own blocks 2g, 2g+1)."""
    return list(range(0, 2 * g + 2)) + list(range(8, 8 + 2 * g + 2))


def build_nc(am_zero=True):
    nc = bacc.Bacc("TRN2", target_bir_lowering=False, debug=False,
                   num_devices=N_CORES)

    X = nc.dram_tensor("X", [S, D], BF16, kind="ExternalInput")
    XQ = nc.dram_tensor("XQ", [OWN, D], F32, kind="ExternalInput")
    MSKE = nc.dram_tensor("MSKE", [2, 128, 512], BF16, kind="ExternalInput")
    EAM = (None if am_zero else
           nc.dram_tensor("EAM", [128, SB], F32, kind="ExternalInput"))
    WQ = nc.dram_tensor("WQ", [D, D], BF16, kind="ExternalInput")
    WK = nc.dram_tensor("WK", [D, D], BF16, kind="ExternalInput")
    WV = nc.dram_tensor("WV", [D, D], BF16, kind="ExternalInput")
    BQ = nc.dram_tensor("BQ", [128, 8], F32, kind="ExternalInput")
    BK = nc.dram_tensor("BK", [128, 8], F32, kind="ExternalInput")
    BV = nc.dram_tensor("BV", [1, D], F32, kind="ExternalInput")
    WP = nc.dram_tensor("WP", [D, D], BF16, kind="ExternalInput")
    WF = nc.dram_tensor("WF", [D, DFF], BF16, kind="ExternalInput")
    BF = nc.dram_tensor("BF", [128, 32], F32, kind="ExternalInput")
    WF2 = nc.dram_tensor("WF2", [DFF, D], BF16, kind="ExternalInput")
    BF2 = nc.dram_tensor("BF2", [1, D], F32, kind="ExternalInput")
    OUT = nc.dram_tensor("OUT", [OWN, D], F32, kind="ExternalOutput")

    with tile.TileContext(nc) as tc:
        _body(nc, tc, X, XQ, MSKE, EAM, WQ, WK, WV, BQ, BK, BV, WP, WF, BF,
              WF2, BF2, OUT, am_zero)
    nc.compile()
    return nc


def _ln_stats(nc, stat, src, eps_t):
    """LN stats of src [128, D] -> (rinv [128,1], nb [128,1]) with
    nb = -mean * rinv."""
    sub = 512
    nsub = D // sub
    xs = src.rearrange("p (n s) -> p n s", s=sub)
    stats = stat.tile([128, nsub, nc.vector.BN_STATS_DIM], F32, tag="bnst")
    for j in range(nsub):
        nc.vector.bn_stats(out=stats[:, j, :], in_=xs[:, j, :])
    mv = stat.tile([128, nc.vector.BN_AGGR_DIM], F32, tag="bnag")
    nc.vector.bn_aggr(out=mv[:, :], in_=stats[:, :, :])
    nc.scalar.activation(out=mv[:, 1:2], in_=mv[:, 1:2], func=AF.Sqrt,
                         bias=eps_t[:], scale=1.0)
    rinv = stat.tile([128, 1], F32, tag="rinv")
    nc.vector.reciprocal(out=rinv[:], in_=mv[:, 1:2])
    nb = stat.tile([128, 1], F32, tag="nb")
    nc.vector.scalar_tensor_tensor(out=nb[:], in0=mv[:, 0:1], scalar=-1.0,
                                   in1=rinv[:], op0=ALU.mult, op1=ALU.mult)
    return rinv, nb


def _body(nc, tc, X, XQ, MSKE, EAM, WQ, WK, WV, BQ, BK, BV, WP, WF, BF,
          WF2, BF2, OUT, am_zero=True):
    PL = int(os.environ.get("KPHASES", "9"))
    with contextlib.ExitStack() as top:
        cst = top.enter_context(tc.tile_pool(name="cst", bufs=1))
        stat = top.enter_context(tc.tile_pool(name="stat", bufs=4))

        ident = cst.tile([128, 128], F32)
        make_identity(nc, ident[:])
        ones_f = cst.tile([1, 128], F32)
        nc.vector.memset(ones_f[:], 1.0)
        ones_c3 = cst.tile([128, HPS, 1], BF16)
        nc.vector.memset(ones_c3[:], 1.0)
        ones_r = cst.tile([1, 128], F32R)   # bias-row lhsT
        nc.scalar.copy(ones_r[:], ones_f[:])
        ones_b = cst.tile([1, 64], F32R)    # denominator-broadcast lhsT
        nc.scalar.copy(ones_b[:], ones_f[:, 0:64])
        eps_t = cst.tile([128, 1], F32)
        nc.vector.memset(eps_t[:], EPS)
        ident_b = cst.tile([128, 128], BF16)
        nc.scalar.copy(ident_b[:], ident[:])

        with contextlib.ExitStack() as attn_stack:
            atp = attn_stack.enter_context(tc.tile_pool(name="atp", bufs=1))
            aT = [atp.tile([128, OWN], BF16, tag=f"aT{p}", name=f"aT{p}")
                  for p in range(8)]

            with contextlib.ExitStack() as ht_stack:
                ht = ht_stack.enter_context(tc.tile_pool(name="ht", bufs=1))
                # hT[db][tg] : [128, 512] bf16, feature-major LN1(x)
                hT = [[ht.tile([128, 512], BF16, tag=f"hT{db}_{tg}",
                               name=f"hT{db}_{tg}") for tg in range(4)]
                      for db in range(8)]

                with contextlib.ExitStack() as hs_stack:
                    # attention-phase pools are created BEFORE the phase-1
                    # pools so phase-1 buffer teardown never aliases them
                    kvq = hs_stack.enter_context(
                        tc.tile_pool(name="kvq", bufs=2))
                    mskp = hs_stack.enter_context(
                        tc.tile_pool(name="mskp", bufs=1))
                    att = hs_stack.enter_context(
                        tc.tile_pool(name="att", bufs=3))
                    wst = hs_stack.enter_context(
                        tc.tile_pool(name="wstA", bufs=2))
                    psKV = hs_stack.enter_context(
                        tc.tile_pool(name="psKV", bufs=2, space="PSUM"))
                    psS = hs_stack.enter_context(
                        tc.tile_pool(name="psS", bufs=2, space="PSUM"))

                    # first X chunk goes to the head of the DMA queue so
                    # LN1 starts as early as possible
                    x_first = mskp.tile([128, 2, D], BF16, tag="xf",
                                        name="xf")
                    nc.sync.dma_start(
                        x_first[:],
                        X[0:256, :].rearrange("(i p) d -> p i d", p=128))

                    # 0/1 exp-masks (bf16) + per-token exp(attn-mask)
                    mskE = mskp.tile([128, 2, 512], BF16, tag="mskE",
                                     name="mskE")
                    nc.sync.dma_start(mskE[:],
                                      MSKE[:, :, :].rearrange("m p f -> p m f"))
                    eam = None
                    if not am_zero:
                        eam = mskp.tile([128, SB], F32, tag="eam", name="eam")
                        nc.sync.dma_start(eam[:], EAM[:, :])
                    bq_t = mskp.tile([128, 8], F32, tag="bq", name="bq")
                    nc.sync.dma_start(bq_t[:], BQ[:, :])
                    bk_t = mskp.tile([128, 8], F32, tag="bk", name="bk")
                    nc.sync.dma_start(bk_t[:], BK[:, :])
                    bv_t = mskp.tile([1, D], F32R, tag="bv", name="bv")
                    nc.sync.dma_start(bv_t[:], BV[:, :].bitcast(F32R))

                    # ---- Phase 1: LN1 over ctx + transpose -> hT ----
                    # 4 token-blocks transpose into one PSUM bank, so each
                    # hT[db][tg] tile is produced by a single wide copy.
                    with tc.tile_pool(name="psT", bufs=2, space="PSUM") \
                            as psT, \
                         tc.tile_pool(name="xin1", bufs=4) as xin, \
                         tc.tile_pool(name="xln", bufs=6) as xlnp:
                        for xg in range(4):
                            xts = []
                            for xh in range(2):
                                if xg == 0 and xh == 0:
                                    xts.append(x_first)
                                    continue
                                x_t = xin.tile([128, 2, D], BF16, tag="x1")
                                nc.sync.dma_start(
                                    x_t[:],
                                    X[xg * 512 + xh * 256:
                                      xg * 512 + (xh + 1) * 256,
                                      :].rearrange("(i p) d -> p i d",
                                                   p=128))
                                xts.append(x_t)
                            xls = []
                            for i in range(4):
                                xv = xts[i // 2][:, i % 2, :]
                                rinv, nb = _ln_stats(nc, stat, xv, eps_t)
                                x_ln = xlnp.tile([128, D], BF16, tag="xln")
                                nc.scalar.activation(out=x_ln[:], in_=xv,
                                                     func=AF.Identity,
                                                     bias=nb[:],
                                                     scale=rinv[:])
                                xls.append(x_ln)
                            for db in range(8):
                                pt = psT.tile([128, 512], BF16, tag="tp")
                                for i in range(4):
                                    nc.tensor.transpose(
                                        pt[:, i * 128:(i + 1) * 128],
                                        xls[i][:, db * 128:(db + 1) * 128],
                                        ident_b[:])
                                if db % 2 == 0:
                                    nc.vector.tensor_copy(hT[db][xg][:],
                                                          pt[:])
                                else:
                                    nc.scalar.copy(hT[db][xg][:], pt[:])

                    if PL < 2:
                        return
                    # psA reuses psT's freed banks; the region-reuse
                    # dependency (first pa write after last phase-1
                    # transpose copy) is subsumed by the data dependency
                    # attention -> K/V -> hT -> phase 1.
                    psA = hs_stack.enter_context(
                        tc.tile_pool(name="psA", bufs=2, space="PSUM"))
                    for hs in range(HSETS):
                        # ---- K/V/Q projections for this head set ----
                        kT = [kvq.tile([128, S], BF16, tag=f"kT{p}",
                                       name=f"kT{p}") for p in range(2)]
                        qT = [kvq.tile([128, OWN], BF16, tag=f"qT{p}",
                                       name=f"qT{p}") for p in range(2)]
                        vS = [kvq.tile([128, HPS, HD + 1], BF16,
                                       tag=f"vS{tb}", name=f"vS{tb}")
                              for tb in range(SB)]

                        wkq = []
                        for p in range(2):
                            fcol = hs * 256 + p * 128
                            wk_t = wst.tile([128, 8, 128], BF16,
                                            tag=f"wk{p}", name=f"wk{p}")
                            nc.sync.dma_start(
                                wk_t[:],
                                WK[:, fcol:fcol + 128].rearrange(
                                    "(i p2) f -> p2 i f", p2=128))
                            wq_t = wst.tile([128, 8, 128], BF16,
                                            tag=f"wq{p}", name=f"wq{p}")
                            nc.sync.dma_start(
                                wq_t[:],
                                WQ[:, fcol:fcol + 128].rearrange(
                                    "(i p2) f -> p2 i f", p2=128))
                            wkq.append((wk_t, wq_t))
                        wv_t = wst.tile([128, 8, 256], BF16, tag="wv",
                                        name="wv")
                        nc.sync.dma_start(
                            wv_t[:],
                            WV[:, hs * 256:(hs + 1) * 256].rearrange(
                                "(i p2) f -> p2 i f", p2=128))

                        for p in range(2):
                            wk_t, wq_t = wkq[p]
                            bcol = hs * 2 + p
                            for tg in range(4):
                                ps = psKV.tile([128, 512], F32, tag="pk")
                                for db in range(8):
                                    nc.tensor.matmul(
                                        ps[:], wk_t[:, db, :], hT[db][tg][:],
                                        start=(db == 0), stop=(db == 7))
                                nc.vector.tensor_scalar_add(
                                    out=kT[p][:, tg * 512:(tg + 1) * 512],
                                    in0=ps[:],
                                    scalar1=bk_t[:, bcol:bcol + 1])
                            for tg in range(2):
                                ps = psKV.tile([128, 512], F32, tag="pk")
                                for db in range(8):
                                    nc.tensor.matmul(
                                        ps[:], wq_t[:, db, :],
                                        hT[db][2 + tg][:],
                                        start=(db == 0), stop=(db == 7))
                                nc.vector.tensor_scalar_add(
                                    out=qT[p][:, tg * 512:(tg + 1) * 512],
                                    in0=ps[:],
                                    scalar1=bq_t[:, bcol:bcol + 1])

                        # V bias broadcast tile [128, 256] for this head set
                        psb = psKV.tile([128, 512], F32, tag="pk")
                        nc.tensor.matmul(
                            psb[:, 0:256], ones_r[:],
                            bv_t[0:1, hs * 256:(hs + 1) * 256],
                            start=True, stop=True)
                        bvb = att.tile([128, HPS, HD], F32, tag="bvb")
                        nc.scalar.copy(
                            bvb[:],
                            psb[:, 0:256].rearrange("p (h d) -> p h d", d=HD))

                        for tb in range(SB):
                            ps = psKV.tile([128, 512], F32, tag="pk")
                            for db in range(8):
                                nc.tensor.matmul(
                                    ps[:, 0:256],
                                    hT[db][tb // 4][:, (tb % 4) * 128:
                                                    (tb % 4 + 1) * 128],
                                    wv_t[:, db, :],
                                    start=(db == 0), stop=(db == 7))
                            nc.vector.tensor_tensor(
                                out=vS[tb][:, :, 0:HD],
                                in0=ps[:, 0:256].rearrange(
                                    "p (h d) -> p h d", d=HD),
                                in1=bvb[:], op=ALU.add)
                            nc.gpsimd.tensor_copy(vS[tb][:, :, HD:HD + 1],
                                                  ones_c3[:])

                        # ---- attention for this head set ----
                        # PV runs as P.T @ V: the exp block [128k, 128q] is
                        # the stationary operand and V [128k, 65] the bf16
                        # moving operand (65 rows/matmul). The ones column
                        # of V accumulates softmax denominators into the
                        # output's col 64, so normalization is a cheap
                        # per-partition scalar multiply; a PE transpose
                        # brings the normalized [q, feat] block back to
                        # feature-major aT for proj.
                        for g in range(NQG):
                            kl = _klist(g)
                            nquads = g + 1
                            for h in range(HPS):
                                p, sub = h // 2, h % 2
                                # one bank: q-sub accumulators at cols
                                # 0:65 / 128:193, transposed-normalized
                                # output at cols 256:512
                                pab = psA.tile([128, 2 * QG], F32, tag="pab")
                                pq = [pab[:, 0:HD + 1],
                                      pab[:, 128:128 + HD + 1]]
                                for qd in range(nquads):
                                    blocks = kl[4 * qd:4 * qd + 4]
                                    pss = psS.tile([128, 1024], F32,
                                                   tag="ps")
                                    for u in range(4):
                                        kb = blocks[u]
                                        nc.tensor.matmul(
                                            pss[:, u * QG:(u + 1) * QG],
                                            kT[p][sub * 64:(sub + 1) * 64,
                                                  kb * 128:(kb + 1) * 128],
                                            qT[p][sub * 64:(sub + 1) * 64,
                                                  g * QG:(g + 1) * QG],
                                            start=True, stop=True)
                                    wide = att.tile([128, 1024], BF16,
                                                    tag="wide", name="wide")
                                    nc.scalar.activation(wide[:], pss[:],
                                                         AF.Exp)
                                    if qd == g // 2:
                                        sl = wide[:, (g % 2) * 512:
                                                  (g % 2) * 512 + 512]
                                        nc.vector.tensor_mul(
                                            sl, sl, mskE[:, 0, :])
                                    if qd == g:
                                        sl = wide[:, 512:1024]
                                        nc.vector.tensor_mul(
                                            sl, sl, mskE[:, 1, :])
                                    if not am_zero:
                                        for u in range(4):
                                            kb = blocks[u]
                                            sl = wide[:, u * QG:(u + 1) * QG]
                                            nc.vector.tensor_scalar_mul(
                                                out=sl, in0=sl,
                                                scalar1=eam[:, kb:kb + 1])
                                    # one accumulation group for the whole
                                    # bank: start clears the bank-wide
                                    # has_written bits, so only the very
                                    # first matmul may carry it
                                    for u in range(4):
                                        kb = blocks[u]
                                        for qs in range(2):
                                            nc.tensor.matmul(
                                                pq[qs],
                                                wide[:, u * QG + qs * 128:
                                                     u * QG + qs * 128
                                                     + 128],
                                                vS[kb][:, h, :],
                                                start=(qd == 0 and u == 0
                                                       and qs == 0),
                                                stop=(qd == nquads - 1
                                                      and u == 3
                                                      and qs == 1),
                                                skip_group_check=True)
                                ap_idx = 2 * hs + p
                                for qs in range(2):
                                    rec = att.tile([128, 1], F32,
                                                   tag="rec")
                                    nc.vector.reciprocal(
                                        rec[:], pq[qs][:, HD:HD + 1])
                                    anrm = att.tile([128, HD], BF16,
                                                    tag="anrm")
                                    nc.vector.tensor_scalar_mul(
                                        out=anrm[:], in0=pq[qs][:, 0:HD],
                                        scalar1=rec[:])
                                    nc.tensor.transpose(
                                        pab[0:HD, QG + qs * 64:
                                            QG + (qs + 1) * 64].bitcast(
                                                BF16),
                                        anrm[:], ident_b[:])
                                dst = aT[ap_idx][sub * 64:(sub + 1) * 64,
                                                 g * QG:(g + 1) * QG]
                                src = pab[0:HD, QG:QG + 128].bitcast(BF16)
                                if h % 2 == 0:
                                    nc.vector.tensor_copy(dst, src)
                                else:
                                    nc.scalar.copy(dst, src)

            if PL < 4:
                return
            # ---- proj + residual -> x2 ; prefetch WF/BF/BF2 ----
            psT2 = top.enter_context(
                tc.tile_pool(name="psT2", bufs=2, space="PSUM"))
            psF = top.enter_context(
                tc.tile_pool(name="psF", bufs=2, space="PSUM"))
            x2p = top.enter_context(tc.tile_pool(name="x2p", bufs=1,
                                                 side="right"))
            wfp = top.enter_context(tc.tile_pool(name="wfp", bufs=1,
                                                 side="right"))

            # proj weights + residual inputs issue FIRST; the long WF
            # prefetch queues behind them on the SP queue
            wstp = attn_stack.enter_context(tc.tile_pool(name="wstP",
                                                         bufs=1))
            wpt = []
            for fg in range(2):
                w_t = wstp.tile([128, 8, 512], BF16, tag=f"wp{fg}",
                                name=f"wp{fg}")
                nc.sync.dma_start(
                    w_t[:],
                    WP[:, fg * 512:(fg + 1) * 512].rearrange(
                        "(i p2) f -> p2 i f", p2=128))
                wpt.append(w_t)
            x2 = [x2p.tile([128, 4, D], F32, tag=f"x2{i}", name=f"x2{i}")
                  for i in range(2)]
            for i in range(2):
                nc.sync.dma_start(
                    x2[i][:],
                    XQ[i * 512:(i + 1) * 512, :].rearrange(
                        "(i2 p) d -> p i2 d", p=128))

            wf_t = [wfp.tile([128, DFF], BF16, tag=f"wf{db}",
                             name=f"wf{db}") for db in range(8)]
            for db in range(8):
                nc.sync.dma_start(wf_t[db][:],
                                  WF[db * 128:(db + 1) * 128, :])
            bf_t = wfp.tile([128, 32], F32, tag="bf", name="bf")
            nc.sync.dma_start(bf_t[:], BF[:, :])
            bf2_t = wfp.tile([1, D], F32R, tag="bf2", name="bf2")
            nc.sync.dma_start(bf2_t[:], BF2[:, :].bitcast(F32R))

            def x2v(tb):
                return x2[tb // 4][:, tb % 4, :]

            with tc.tile_pool(name="psP", bufs=2, space="PSUM") as psP:
                for tb in range(OB):
                    for fg in range(2):
                        ps = psP.tile([128, 512], F32, tag="pp")
                        for ab in range(8):
                            nc.tensor.matmul(
                                ps[:], aT[ab][:, tb * 128:(tb + 1) * 128],
                                wpt[fg][:, ab, :], start=(ab == 0),
                                stop=(ab == 7))
                        dst = x2v(tb)[:, fg * 512:(fg + 1) * 512]
                        nc.vector.tensor_tensor(out=dst, in0=dst, in1=ps[:],
                                                op=ALU.add)

        if PL < 5:
            return
        # ---- LN2 + transpose -> h2T ; then MLP ----
        with contextlib.ExitStack() as mlp_stack:
            ht2 = mlp_stack.enter_context(tc.tile_pool(name="ht2", bufs=1))
            h2T = [[ht2.tile([128, 512], BF16, tag=f"h2T{db}_{tg}",
                             name=f"h2T{db}_{tg}") for tg in range(2)]
                   for db in range(8)]
            with tc.tile_pool(name="xln2", bufs=6) as xlnp:
                for tg in range(2):
                    xls = []
                    for i in range(4):
                        tb = tg * 4 + i
                        rinv, nb = _ln_stats(nc, stat, x2v(tb), eps_t)
                        x_ln = xlnp.tile([128, D], BF16, tag="xln")
                        nc.scalar.activation(out=x_ln[:], in_=x2v(tb),
                                             func=AF.Identity,
                                             bias=nb[:], scale=rinv[:])
                        xls.append(x_ln)
                    for db in range(8):
                        pt = psT2.tile([128, 512], BF16, tag="tp")
                        for i in range(4):
                            nc.tensor.transpose(
                                pt[:, i * 128:(i + 1) * 128],
                                xls[i][:, db * 128:(db + 1) * 128],
                                ident_b[:])
                        if db % 2 == 0:
                            nc.vector.tensor_copy(h2T[db][tg][:], pt[:])
                        else:
                            nc.scalar.copy(h2T[db][tg][:], pt[:])

            if PL < 6:
                return
            with contextlib.ExitStack() as mlp2:
                gtp = mlp2.enter_context(tc.tile_pool(name="gtp", bufs=1))
                wst6 = mlp2.enter_context(tc.tile_pool(name="wstF6", bufs=4))
                outp = mlp2.enter_context(tc.tile_pool(name="outp", bufs=3))
                psO = None
                for tg in range(2):
                    gt = [gtp.tile([128, 512], BF16, tag=f"gt{j}",
                                   name=f"gt{j}") for j in range(32)]
                    for j in range(32):
                        ps = psF.tile([128, 512], F32, tag="pf")
                        for db in range(8):
                            nc.tensor.matmul(
                                ps[:], wf_t[db][:, j * 128:(j + 1) * 128],
                                h2T[db][tg][:],
                                start=(db == 0), stop=(db == 7))
                        nc.scalar.activation(gt[j][:], ps[:],
                                             AF.Gelu_apprx_tanh,
                                             bias=bf_t[:, j:j + 1], scale=1.0)
                    if psO is None:
                        psO = mlp2.enter_context(
                            tc.tile_pool(name="psO", bufs=1, space="PSUM"))
                    for fg in range(2):
                        last = (tg == 1 and fg == 1)
                        pso = [psO.tile([128, 512], F32, tag=f"po{tb}",
                                        name=f"po{tb}") for tb in range(4)]
                        w8s = []
                        for jj in range(4):
                            w8 = wst6.tile([128, 8, 512], BF16, tag="wf2",
                                           name="wf2")
                            nc.sync.dma_start(
                                w8[:],
                                WF2[jj * 1024:(jj + 1) * 1024,
                                    fg * 512:(fg + 1) * 512].rearrange(
                                        "(i p2) f -> p2 i f", p2=128))
                            w8s.append(w8)
                            if last:
                                continue
                            for jr in range(8):
                                j = jj * 8 + jr
                                for tb in range(4):
                                    nc.tensor.matmul(
                                        pso[tb][:],
                                        gt[j][:, tb * 128:(tb + 1) * 128],
                                        w8[:, jr, :], start=(j == 0),
                                        stop=False)
                        for tb in range(4):
                            if last:
                                # tb-major on the final pass: each output
                                # block drains (bias/add/store) while the
                                # next accumulates, hiding the tail chain
                                for jj in range(4):
                                    for jr in range(8):
                                        j = jj * 8 + jr
                                        nc.tensor.matmul(
                                            pso[tb][:],
                                            gt[j][:, tb * 128:
                                                  (tb + 1) * 128],
                                            w8s[jj][:, jr, :],
                                            start=(j == 0), stop=False)
                            nc.tensor.matmul(
                                pso[tb][:], ones_r[:],
                                bf2_t[0:1, fg * 512:(fg + 1) * 512],
                                start=False, stop=True)
                            gtb = tg * 4 + tb
                            o_t = outp.tile([128, 512], F32, tag="ot")
                            nc.vector.tensor_add(
                                o_t[:], pso[tb][:],
                                x2v(gtb)[:, fg * 512:(fg + 1) * 512])
                            nc.scalar.dma_start(
                                OUT[gtb * 128:(gtb + 1) * 128,
                                    fg * 512:(fg + 1) * 512], o_t[:])


_NC_CACHE = {}


def _get_nc(am_zero=True):
    key = f"nc{int(am_zero)}"
    if key not in _NC_CACHE:
        _NC_CACHE[key] = build_nc(am_zero)
    return _NC_CACHE[key]


def _perm_for(f):
    other = [2 * j + (1 - f) for j in range(8)]
    own = [2 * j + f for j in range(8)]
    blocks = other + own
    return np.concatenate([np.arange(b * 128, (b + 1) * 128) for b in blocks])


def make_in_maps(hidden_states, attention_mask, ln1_g, ln1_b, W_attn, b_attn,
                 W_proj, b_proj, ln2_g, ln2_b, W_fc, b_fc, W_fc2, b_fc2):
    f32 = lambda a: np.asarray(a, dtype=np.float32)
    bf16 = lambda a: np.ascontiguousarray(a.astype(ml_dtypes.bfloat16))
    hidden_states = f32(hidden_states)
    attention_mask = f32(attention_mask)
    ln1_g, ln1_b = f32(ln1_g), f32(ln1_b)
    ln2_g, ln2_b = f32(ln2_g), f32(ln2_b)
    W_attn, b_attn = f32(W_attn), f32(b_attn)
    W_proj, b_proj = f32(W_proj), f32(b_proj)
    W_fc, b_fc = f32(W_fc), f32(b_fc)
    W_fc2, b_fc2 = f32(W_fc2), f32(b_fc2)

    # Fold LN affines into the consuming matmuls (exact algebra, fp64 on host).
    Wa_eff = (ln1_g.astype(np.float64)[:, None] * W_attn).astype(np.float32)
    ba_eff = (b_attn.astype(np.float64)
              + ln1_b.astype(np.float64) @ W_attn).astype(np.float32)
    scale = 1.0 / np.sqrt(np.float32(HD))
    WQn = (Wa_eff[:, 0:D] * scale).astype(np.float32)
    BQn = (ba_eff[0:D] * scale).astype(np.float32)
    WKn, BKn = Wa_eff[:, D:2 * D].copy(), ba_eff[D:2 * D].copy()
    WVn, BVn = Wa_eff[:, 2 * D:3 * D].copy(), ba_eff[2 * D:3 * D].copy()
    Wf_eff = (ln2_g.astype(np.float64)[:, None] * W_fc).astype(np.float32)
    bf_eff = (b_fc.astype(np.float64)
              + ln2_b.astype(np.float64) @ W_fc).astype(np.float32)

    shared = {
        "WQ": bf16(WQn),
        "WK": bf16(WKn),
        "WV": bf16(WVn),
        "BQ": np.ascontiguousarray(BQn.reshape(8, 128).T),
        "BK": np.ascontiguousarray(BKn.reshape(8, 128).T),
        "BV": np.ascontiguousarray(BVn[None, :]),
        "WP": bf16(W_proj),
        "WF": bf16(Wf_eff),
        "BF": np.ascontiguousarray(bf_eff.reshape(32, 128).T),
        "WF2": bf16(W_fc2),
        "BF2": np.ascontiguousarray(b_fc2[None, :]),
    }

    am_zero = bool(np.all(attention_mask == 0))
    in_maps, perms = [], []
    for c in range(N_CORES):
        b, f = c >> 1, c & 1
        perm = _perm_for(f)
        perms.append(perm)
        x_ctx = np.ascontiguousarray(hidden_states[b][perm])
        xq = np.ascontiguousarray(hidden_states[b][perm[OWN:]]
                                  + b_proj[None, :])
        gk = perm
        gq = perm[OWN:]
        live = (gk[:, None] <= gq[None, :]).astype(np.float32)
        # 0/1 exp-masks: [:, u*QG:(u+1)*QG] is k-block (base+u) vs q-group 0
        # pair 0: other-parity blocks (0, 1); pair 1: own blocks (8, 9).
        # The relative pattern is g-independent.
        msk = np.zeros((2, 128, 512), np.float32)
        for u, j in enumerate([0, 1]):
            msk[0, :, u * QG:(u + 1) * QG] = live[
                j * 128:(j + 1) * 128, 0:QG]
        for u, j in enumerate([8, 9]):
            msk[1, :, u * QG:(u + 1) * QG] = live[
                j * 128:(j + 1) * 128, 0:QG]
        im = {"X": bf16(x_ctx), "XQ": xq, "MSKE": bf16(msk), **shared}
        if not am_zero:
            am = attention_mask[b, 0, 0, :].astype(np.float64)
            eam = np.exp(am[perm]).astype(np.float32)
            im["EAM"] = np.ascontiguousarray(eam.reshape(SB, 128).T)
        in_maps.append(im)
    return in_maps, perms, am_zero


def kernel(hidden_states, attention_mask, ln1_g, ln1_b, W_attn, b_attn,
           W_proj, b_proj, ln2_g, ln2_b, W_fc, b_fc, W_fc2, b_fc2):
    in_maps, perms, am_zero = make_in_maps(
        hidden_states, attention_mask, ln1_g, ln1_b, W_attn, b_attn,
        W_proj, b_proj, ln2_g, ln2_b, W_fc, b_fc, W_fc2, b_fc2)
    nc = _get_nc(am_zero)
    res = run_bass_kernel_spmd(nc, in_maps, core_ids=list(range(N_CORES)))
    out = np.empty((B, S, D), dtype=np.float32)
    for c in range(N_CORES):
        b = c >> 1
        out[b][perms[c][OWN:]] = res.results[c]["OUT"]
    return out



# revision 11
# speedup vs baseline: 1.0579x; 1.0579x over previous
"""Fused GPT-2 transformer block on 8 Trainium2 NeuronCores.

Sharding: 8 cores = 4 batches x 2 causal-balanced folds. Core (b, f) owns the 8
interleaved 128-token blocks of parity f of batch b (queries), and receives all
2048 tokens of batch b as context, permuted [other-parity blocks | own blocks].
Causality is enforced by exact 0/1 mask multiplies after exp, so a single SPMD
program serves all cores. No collectives.

Layouts: LN1(x) is PE-transposed to hT [D, tok] (bf16); Q/K are produced in
head-major transposed layout (bf16), V in token-major layout with an appended
ones column (so the P@V matmul also accumulates softmax denominators).
Exp runs on the scalar engine in [128,1024] slabs straight from PSUM to bf16;
causal masking is a 0/1 elementwise multiply on the vector engine afterwards
(exp(s+m) == exp(s)*exp(m) with exp(m) in {0,1} exactly). proj/fc matmuls
contract against feature-major lhsT slices. All weights travel as bf16;
LN affines and the proj bias are folded on the host.
"""

import contextlib
import os

import numpy as np
import ml_dtypes

import concourse.bass as bass
import concourse.mybir as mybir
import concourse.tile as tile
from concourse import bacc
from concourse.bass_utils import run_bass_kernel_spmd
from concourse.masks import make_identity

F32 = mybir.dt.float32
F32R = mybir.dt.float32r
BF16 = mybir.dt.bfloat16
F8 = mybir.dt.float8e4
DR = mybir.MatmulPerfMode.DoubleRow
AF = mybir.ActivationFunctionType
ALU = mybir.AluOpType

B, S, D, H = 4, 2048, 1024, 16
HD = D // H          # 64
DFF = 4 * D          # 4096
EPS = 1e-5
MASKED_BIAS = -10000.0
N_CORES = 8

SB = S // 128        # 16 ctx blocks
OWN = S // 2         # 1024 own tokens
OB = OWN // 128      # 8 own blocks
NQG = 4              # q-groups of 256
QG = 256
HSETS = 4            # head sets
HPS = H // HSETS     # 4 heads per set


def _klist(g):
    """ctx k-block indices computed for q-group g (own blocks 2g, 2g+1)."""
    return list(range(0, 2 * g + 2)) + list(range(8, 8 + 2 * g + 2))


def build_nc(am_zero=True):
    nc = bacc.Bacc("TRN2", target_bir_lowering=False, debug=False,
                   num_devices=N_CORES)

    X = nc.dram_tensor("X", [S, D], BF16, kind="ExternalInput")
    XQ = nc.dram_tensor("XQ", [OWN, D], F32, kind="ExternalInput")
    MSKE = nc.dram_tensor("MSKE", [2, 128, 512], BF16, kind="ExternalInput")
    EAM = (None if am_zero else
           nc.dram_tensor("EAM", [128, SB], F32, kind="ExternalInput"))
    WQ = (nc.dram_tensor("WQ", [D, D], F8, kind="ExternalInput"),
          nc.dram_tensor("WQR", [D, D], F8, kind="ExternalInput"))
    WK = (nc.dram_tensor("WK", [D, D], F8, kind="ExternalInput"),
          nc.dram_tensor("WKR", [D, D], F8, kind="ExternalInput"))
    WV = (nc.dram_tensor("WV", [D, D], F8, kind="ExternalInput"),
          nc.dram_tensor("WVR", [D, D], F8, kind="ExternalInput"))
    BQ = nc.dram_tensor("BQ", [128, 8], F32, kind="ExternalInput")
    WP = (nc.dram_tensor("WP", [D, D], F8, kind="ExternalInput"),
          nc.dram_tensor("WPR", [D, D], F8, kind="ExternalInput"))
    WF = nc.dram_tensor("WF", [D, DFF], F8, kind="ExternalInput")
    WFR = nc.dram_tensor("WFR", [D, DFF], F8, kind="ExternalInput")
    BF = nc.dram_tensor("BF", [128, 32], F32, kind="ExternalInput")
    WF2 = nc.dram_tensor("WF2", [DFF, D], F8, kind="ExternalInput")
    WF2R = nc.dram_tensor("WF2R", [DFF, D], F8, kind="ExternalInput")
    BF2 = nc.dram_tensor("BF2", [1, D], F32, kind="ExternalInput")
    OUT = nc.dram_tensor("OUT", [OWN, D], F32, kind="ExternalOutput")

    with tile.TileContext(nc) as tc:
        _body(nc, tc, X, XQ, MSKE, EAM, WQ, WK, WV, BQ, BK, BV, WP, WF, WFR,
              BF, WF2, WF2R, BF2, OUT, am_zero)
    nc.compile()
    return nc


def _ln_stats(nc, stat, src, eps_t):
    """LN stats of src [128, D] -> (rinv [128,1], nb [128,1]) with
    nb = -mean * rinv."""
    sub = 512
    nsub = D // sub
    xs = src.rearrange("p (n s) -> p n s", s=sub)
    stats = stat.tile([128, nsub, nc.vector.BN_STATS_DIM], F32, tag="bnst")
    for j in range(nsub):
        nc.vector.bn_stats(out=stats[:, j, :], in_=xs[:, j, :])
    mv = stat.tile([128, nc.vector.BN_AGGR_DIM], F32, tag="bnag")
    nc.vector.bn_aggr(out=mv[:, :], in_=stats[:, :, :])
    nc.scalar.activation(out=mv[:, 1:2], in_=mv[:, 1:2], func=AF.Sqrt,
                         bias=eps_t[:], scale=1.0)
    rinv = stat.tile([128, 1], F32, tag="rinv")
    nc.vector.reciprocal(out=rinv[:], in_=mv[:, 1:2])
    nb = stat.tile([128, 1], F32, tag="nb")
    nc.vector.scalar_tensor_tensor(out=nb[:], in0=mv[:, 0:1], scalar=-1.0,
                                   in1=rinv[:], op0=ALU.mult, op1=ALU.mult)
    return rinv, nb


def _body(nc, tc, X, XQ, MSKE, EAM, WQ, WK, WV, BQ, BK, BV, WP, WF, WFR,
          BF, WF2, WF2R, BF2, OUT, am_zero=True):
    PL = int(os.environ.get("KPHASES", "9"))
    with contextlib.ExitStack() as top:
        cst = top.enter_context(tc.tile_pool(name="cst", bufs=1))
        stat = top.enter_context(tc.tile_pool(name="stat", bufs=4))

        ident = cst.tile([128, 128], F32)
        make_identity(nc, ident[:])
        ones_f = cst.tile([1, 128], F32)
        nc.vector.memset(ones_f[:], 1.0)
        ones_c3 = cst.tile([128, HPS, 1], BF16)
        nc.vector.memset(ones_c3[:], 1.0)
        ones_r = cst.tile([1, 128], F32R)   # bias-row lhsT
        nc.scalar.copy(ones_r[:], ones_f[:])
        ones_b = cst.tile([1, 64], F32R)    # denominator-broadcast lhsT
        nc.scalar.copy(ones_b[:], ones_f[:, 0:64])
        eps_t = cst.tile([128, 1], F32)
        nc.vector.memset(eps_t[:], EPS)
        ident_b = cst.tile([128, 128], BF16)
        nc.scalar.copy(ident_b[:], ident[:])

        with contextlib.ExitStack() as attn_stack:
            atp = attn_stack.enter_context(tc.tile_pool(name="atp", bufs=1))
            aT = [atp.tile([128, OWN], BF16, tag=f"aT{p}", name=f"aT{p}")
                  for p in range(8)]

            with contextlib.ExitStack() as ht_stack:
                ht = ht_stack.enter_context(tc.tile_pool(name="ht", bufs=1))
                # hT[db][tg] : [128, 512] bf16, feature-major LN1(x)
                hT = [[ht.tile([128, 512], BF16, tag=f"hT{db}_{tg}",
                               name=f"hT{db}_{tg}") for tg in range(4)]
                      for db in range(8)]

                with contextlib.ExitStack() as hs_stack:
                    # attention-phase pools are created BEFORE the phase-1
                    # pools so phase-1 buffer teardown never aliases them
                    kvq = hs_stack.enter_context(
                        tc.tile_pool(name="kvq", bufs=2))
                    mskp = hs_stack.enter_context(
                        tc.tile_pool(name="mskp", bufs=1))
                    att = hs_stack.enter_context(
                        tc.tile_pool(name="att", bufs=3))
                    wst = hs_stack.enter_context(
                        tc.tile_pool(name="wstA", bufs=2))
                    psKV = hs_stack.enter_context(
                        tc.tile_pool(name="psKV", bufs=2, space="PSUM"))
                    psS = hs_stack.enter_context(
                        tc.tile_pool(name="psS", bufs=2, space="PSUM"))

                    # first X chunk goes to the head of the DMA queue so
                    # LN1 starts as early as possible
                    x_first = mskp.tile([128, 2, D], BF16, tag="xf",
                                        name="xf")
                    nc.sync.dma_start(
                        x_first[:],
                        X[0:256, :].rearrange("(i p) d -> p i d", p=128))

                    # 0/1 exp-masks (bf16) + per-token exp(attn-mask)
                    mskE = mskp.tile([128, 2, 512], BF16, tag="mskE",
                                     name="mskE")
                    nc.sync.dma_start(mskE[:],
                                      MSKE[:, :, :].rearrange("m p f -> p m f"))
                    eam = None
                    if not am_zero:
                        eam = mskp.tile([128, SB], F32, tag="eam", name="eam")
                        nc.sync.dma_start(eam[:], EAM[:, :])
                    bq_t = mskp.tile([128, 8], F32, tag="bq", name="bq")
                    nc.sync.dma_start(bq_t[:], BQ[:, :])
                    bk_t = mskp.tile([128, 8], F32, tag="bk", name="bk")
                    nc.sync.dma_start(bk_t[:], BK[:, :])
                    bv_t = mskp.tile([1, D], F32R, tag="bv", name="bv")
                    nc.sync.dma_start(bv_t[:], BV[:, :].bitcast(F32R))

                    # ---- Phase 1: LN1 over ctx + transpose -> hT ----
                    # 4 token-blocks transpose into one PSUM bank, so each
                    # hT[db][tg] tile is produced by a single wide copy.
                    with tc.tile_pool(name="psT", bufs=2, space="PSUM") \
                            as psT, \
                         tc.tile_pool(name="xin1", bufs=4) as xin, \
                         tc.tile_pool(name="xln", bufs=6) as xlnp:
                        for xg in range(4):
                            xts = []
                            for xh in range(2):
                                if xg == 0 and xh == 0:
                                    xts.append(x_first)
                                    continue
                                x_t = xin.tile([128, 2, D], BF16, tag="x1")
                                nc.sync.dma_start(
                                    x_t[:],
                                    X[xg * 512 + xh * 256:
                                      xg * 512 + (xh + 1) * 256,
                                      :].rearrange("(i p) d -> p i d",
                                                   p=128))
                                xts.append(x_t)
                            xls = []
                            for i in range(4):
                                xv = xts[i // 2][:, i % 2, :]
                                rinv, nb = _ln_stats(nc, stat, xv, eps_t)
                                x_ln = xlnp.tile([128, D], BF16, tag="xln")
                                nc.scalar.activation(out=x_ln[:], in_=xv,
                                                     func=AF.Identity,
                                                     bias=nb[:],
                                                     scale=rinv[:])
                                xls.append(x_ln)
                            for db in range(8):
                                pt = psT.tile([128, 512], BF16, tag="tp")
                                for i in range(4):
                                    nc.tensor.transpose(
                                        pt[:, i * 128:(i + 1) * 128],
                                        xls[i][:, db * 128:(db + 1) * 128],
                                        ident_b[:])
                                if db % 2 == 0:
                                    nc.vector.tensor_copy(hT[db][xg][:],
                                                          pt[:])
                                else:
                                    nc.scalar.copy(hT[db][xg][:], pt[:])

                    if PL < 2:
                        return
                    # psA reuses psT's freed banks; the region-reuse
                    # dependency (first pa write after last phase-1
                    # transpose copy) is subsumed by the data dependency
                    # attention -> K/V -> hT -> phase 1.
                    psA = hs_stack.enter_context(
                        tc.tile_pool(name="psA", bufs=2, space="PSUM"))
                    for hs in range(HSETS):
                        # ---- K/V/Q projections for this head set ----
                        kT = [kvq.tile([128, S], BF16, tag=f"kT{p}",
                                       name=f"kT{p}") for p in range(2)]
                        qT = [kvq.tile([128, OWN], BF16, tag=f"qT{p}",
                                       name=f"qT{p}") for p in range(2)]
                        vS = [kvq.tile([128, HPS, HD + 1], BF16,
                                       tag=f"vS{tb}", name=f"vS{tb}")
                              for tb in range(SB)]

                        wkq = []
                        for p in range(2):
                            fcol = hs * 256 + p * 128
                            wk_t = wst.tile([128, 8, 128], BF16,
                                            tag=f"wk{p}", name=f"wk{p}")
                            nc.sync.dma_start(
                                wk_t[:],
                                WK[:, fcol:fcol + 128].rearrange(
                                    "(i p2) f -> p2 i f", p2=128))
                            wq_t = wst.tile([128, 8, 128], BF16,
                                            tag=f"wq{p}", name=f"wq{p}")
                            nc.sync.dma_start(
                                wq_t[:],
                                WQ[:, fcol:fcol + 128].rearrange(
                                    "(i p2) f -> p2 i f", p2=128))
                            wkq.append((wk_t, wq_t))
                        wv_t = wst.tile([128, 8, 256], BF16, tag="wv",
                                        name="wv")
                        nc.sync.dma_start(
                            wv_t[:],
                            WV[:, hs * 256:(hs + 1) * 256].rearrange(
                                "(i p2) f -> p2 i f", p2=128))

                        for p in range(2):
                            wk_t, wq_t = wkq[p]
                            bcol = hs * 2 + p
                            for tg in range(4):
                                ps = psKV.tile([128, 512], F32, tag="pk")
                                for db in range(8):
                                    nc.tensor.matmul(
                                        ps[:], wk_t[:, db, :], hT[db][tg][:],
                                        start=(db == 0), stop=(db == 7))
                                nc.vector.tensor_scalar_add(
                                    out=kT[p][:, tg * 512:(tg + 1) * 512],
                                    in0=ps[:],
                                    scalar1=bk_t[:, bcol:bcol + 1])
                            for tg in range(2):
                                ps = psKV.tile([128, 512], F32, tag="pk")
                                for db in range(8):
                                    nc.tensor.matmul(
                                        ps[:], wq_t[:, db, :],
                                        hT[db][2 + tg][:],
                                        start=(db == 0), stop=(db == 7))
                                nc.vector.tensor_scalar_add(
                                    out=qT[p][:, tg * 512:(tg + 1) * 512],
                                    in0=ps[:],
                                    scalar1=bq_t[:, bcol:bcol + 1])

                        # V bias broadcast tile [128, 256] for this head set
                        psb = psKV.tile([128, 512], F32, tag="pk")
                        nc.tensor.matmul(
                            psb[:, 0:256], ones_r[:],
                            bv_t[0:1, hs * 256:(hs + 1) * 256],
                            start=True, stop=True)
                        bvb = att.tile([128, HPS, HD], F32, tag="bvb")
                        nc.scalar.copy(
                            bvb[:],
                            psb[:, 0:256].rearrange("p (h d) -> p h d", d=HD))

                        for tb in range(SB):
                            ps = psKV.tile([128, 512], F32, tag="pk")
                            for db in range(8):
                                nc.tensor.matmul(
                                    ps[:, 0:256],
                                    hT[db][tb // 4][:, (tb % 4) * 128:
                                                    (tb % 4 + 1) * 128],
                                    wv_t[:, db, :],
                                    start=(db == 0), stop=(db == 7))
                            nc.vector.tensor_tensor(
                                out=vS[tb][:, :, 0:HD],
                                in0=ps[:, 0:256].rearrange(
                                    "p (h d) -> p h d", d=HD),
                                in1=bvb[:], op=ALU.add)
                            nc.gpsimd.tensor_copy(vS[tb][:, :, HD:HD + 1],
                                                  ones_c3[:])

                        # ---- attention for this head set ----
                        # PV runs as P.T @ V: the exp block [128k, 128q] is
                        # the stationary operand and V [128k, 65] the bf16
                        # moving operand (65 rows/matmul). The ones column
                        # of V accumulates softmax denominators into the
                        # output's col 64, so normalization is a cheap
                        # per-partition scalar multiply; a PE transpose
                        # brings the normalized [q, feat] block back to
                        # feature-major aT for proj.
                        for g in range(NQG):
                            kl = _klist(g)
                            nquads = g + 1
                            for h in range(HPS):
                                p, sub = h // 2, h % 2
                                # one bank: q-sub accumulators at cols
                                # 0:65 / 128:193, transposed-normalized
                                # output at cols 256:512
                                pab = psA.tile([128, 2 * QG], F32, tag="pab")
                                pq = [pab[:, 0:HD + 1],
                                      pab[:, 128:128 + HD + 1]]
                                for qd in range(nquads):
                                    blocks = kl[4 * qd:4 * qd + 4]
                                    pss = psS.tile([128, 1024], F32,
                                                   tag="ps")
                                    for u in range(4):
                                        kb = blocks[u]
                                        nc.tensor.matmul(
                                            pss[:, u * QG:(u + 1) * QG],
                                            kT[p][sub * 64:(sub + 1) * 64,
                                                  kb * 128:(kb + 1) * 128],
                                            qT[p][sub * 64:(sub + 1) * 64,
                                                  g * QG:(g + 1) * QG],
                                            start=True, stop=True)
                                    wide = att.tile([128, 1024], BF16,
                                                    tag="wide", name="wide")
                                    nc.scalar.activation(wide[:], pss[:],
                                                         AF.Exp)
                                    if qd == g // 2:
                                        sl = wide[:, (g % 2) * 512:
                                                  (g % 2) * 512 + 512]
                                        nc.vector.tensor_mul(
                                            sl, sl, mskE[:, 0, :])
                                    if qd == g:
                                        sl = wide[:, 512:1024]
                                        nc.vector.tensor_mul(
                                            sl, sl, mskE[:, 1, :])
                                    if not am_zero:
                                        for u in range(4):
                                            kb = blocks[u]
                                            sl = wide[:, u * QG:(u + 1) * QG]
                                            nc.vector.tensor_scalar_mul(
                                                out=sl, in0=sl,
                                                scalar1=eam[:, kb:kb + 1])
                                    # one accumulation group for the whole
                                    # bank: start clears the bank-wide
                                    # has_written bits, so only the very
                                    # first matmul may carry it
                                    for u in range(4):
                                        kb = blocks[u]
                                        for qs in range(2):
                                            nc.tensor.matmul(
                                                pq[qs],
                                                wide[:, u * QG + qs * 128:
                                                     u * QG + qs * 128
                                                     + 128],
                                                vS[kb][:, h, :],
                                                start=(qd == 0 and u == 0
                                                       and qs == 0),
                                                stop=(qd == nquads - 1
                                                      and u == 3
                                                      and qs == 1),
                                                skip_group_check=True)
                                ap_idx = 2 * hs + p
                                for qs in range(2):
                                    rec = att.tile([128, 1], F32,
                                                   tag="rec")
                                    nc.vector.reciprocal(
                                        rec[:], pq[qs][:, HD:HD + 1])
                                    anrm = att.tile([128, HD], BF16,
                                                    tag="anrm")
                                    nc.vector.tensor_scalar_mul(
                                        out=anrm[:], in0=pq[qs][:, 0:HD],
                                        scalar1=rec[:])
                                    nc.tensor.transpose(
                                        pab[0:HD, QG + qs * 64:
                                            QG + (qs + 1) * 64].bitcast(
                                                BF16),
                                        anrm[:], ident_b[:])
                                dst = aT[ap_idx][sub * 64:(sub + 1) * 64,
                                                 g * QG:(g + 1) * QG]
                                src = pab[0:HD, QG:QG + 128].bitcast(BF16)
                                if h % 2 == 0:
                                    nc.vector.tensor_copy(dst, src)
                                else:
                                    nc.scalar.copy(dst, src)

            if PL < 4:
                return
            # ---- proj + residual -> x2 ; prefetch WF/BF/BF2 ----
            psT2 = top.enter_context(
                tc.tile_pool(name="psT2", bufs=2, space="PSUM"))
            psF = top.enter_context(
                tc.tile_pool(name="psF", bufs=2, space="PSUM"))
            x2p = top.enter_context(tc.tile_pool(name="x2p", bufs=1,
                                                 side="right"))
            wfp = top.enter_context(tc.tile_pool(name="wfp", bufs=1,
                                                 side="right"))

            # proj weights + residual inputs issue FIRST; the long WF
            # prefetch queues behind them on the SP queue
            wstp = attn_stack.enter_context(tc.tile_pool(name="wstP",
                                                         bufs=1))
            wpt = []
            for fg in range(2):
                w_t = wstp.tile([128, 8, 512], BF16, tag=f"wp{fg}",
                                name=f"wp{fg}")
                nc.sync.dma_start(
                    w_t[:],
                    WP[:, fg * 512:(fg + 1) * 512].rearrange(
                        "(i p2) f -> p2 i f", p2=128))
                wpt.append(w_t)
            x2 = [x2p.tile([128, 4, D], F32, tag=f"x2{i}", name=f"x2{i}")
                  for i in range(2)]
            for i in range(2):
                nc.sync.dma_start(
                    x2[i][:],
                    XQ[i * 512:(i + 1) * 512, :].rearrange(
                        "(i2 p) d -> p i2 d", p=128))

            wf8_t = wfp.tile([128, 8, DFF], F8, tag="wf8", name="wf8")
            wfr8_t = wfp.tile([128, 8, DFF], F8, tag="wfr8", name="wfr8")
            for db in range(8):
                nc.sync.dma_start(wf8_t[:, db, :],
                                  WF[db * 128:(db + 1) * 128, :])
                nc.sync.dma_start(wfr8_t[:, db, :],
                                  WFR[db * 128:(db + 1) * 128, :])
            bf_t = wfp.tile([128, 32], F32, tag="bf", name="bf")
            nc.sync.dma_start(bf_t[:], BF[:, :])
            bf2_t = wfp.tile([1, D], F32R, tag="bf2", name="bf2")
            nc.sync.dma_start(bf2_t[:], BF2[:, :].bitcast(F32R))

            def x2v(tb):
                return x2[tb // 4][:, tb % 4, :]

            with tc.tile_pool(name="psP", bufs=2, space="PSUM") as psP:
                for tb in range(OB):
                    for fg in range(2):
                        ps = psP.tile([128, 512], F32, tag="pp")
                        for ab in range(8):
                            nc.tensor.matmul(
                                ps[:], aT[ab][:, tb * 128:(tb + 1) * 128],
                                wpt[fg][:, ab, :], start=(ab == 0),
                                stop=(ab == 7))
                        dst = x2v(tb)[:, fg * 512:(fg + 1) * 512]
                        nc.vector.tensor_tensor(out=dst, in0=dst, in1=ps[:],
                                                op=ALU.add)

        if PL < 5:
            return
        # ---- LN2 + transpose -> h2T8 (+ fp8 residual) ; then MLP ----
        with contextlib.ExitStack() as mlp_stack:
            ht2 = mlp_stack.enter_context(tc.tile_pool(name="ht2", bufs=1))
            # fp8 dual representation of LN2(x2).T: h2T8 ~ 8*h2, h2Te8 the
            # quantization remainder (tri-fp8 DoubleRow matmul operands)
            h2T8 = [ht2.tile([128, 8, 512], F8, tag=f"h2T8_{tg}",
                             name=f"h2T8_{tg}") for tg in range(2)]
            h2Te8 = [ht2.tile([128, 8, 512], F8, tag=f"h2Te8_{tg}",
                              name=f"h2Te8_{tg}") for tg in range(2)]
            with tc.tile_pool(name="xln2", bufs=6) as xlnp:
                for tg in range(2):
                    xls = []
                    for i in range(4):
                        tb = tg * 4 + i
                        rinv, nb = _ln_stats(nc, stat, x2v(tb), eps_t)
                        x_ln = xlnp.tile([128, D], BF16, tag="xln")
                        nc.scalar.activation(out=x_ln[:], in_=x2v(tb),
                                             func=AF.Identity,
                                             bias=nb[:], scale=rinv[:])
                        xls.append(x_ln)
                    for db in range(8):
                        pt = psT2.tile([128, 512], BF16, tag="tp")
                        for i in range(4):
                            nc.tensor.transpose(
                                pt[:, i * 128:(i + 1) * 128],
                                xls[i][:, db * 128:(db + 1) * 128],
                                ident_b[:])
                        nc.scalar.activation(
                            out=h2T8[tg][:, db, :], in_=pt[:],
                            func=AF.Copy, scale=8.0)
                        nc.vector.scalar_tensor_tensor(
                            out=h2Te8[tg][:, db, :], in0=pt[:], scalar=8.0,
                            in1=h2T8[tg][:, db, :], op0=ALU.mult,
                            op1=ALU.subtract)

            if PL < 6:
                return
            with contextlib.ExitStack() as mlp2:
                gtp = mlp2.enter_context(tc.tile_pool(name="gtp", bufs=1))
                gtbp = mlp2.enter_context(tc.tile_pool(name="gtb", bufs=4))
                wst6 = mlp2.enter_context(tc.tile_pool(name="wstF6", bufs=4))
                outp = mlp2.enter_context(tc.tile_pool(name="outp", bufs=3))
                psO = None

                def fc2_tri(pso_t, jj, pr, tb, gt8, ge8, w2a, w2r):
                    jp = jj * 4 + pr
                    j0 = 2 * jp
                    lhA = gt8[:, j0:j0 + 2, tb * 128:(tb + 1) * 128]
                    lhE = ge8[:, j0:j0 + 2, tb * 128:(tb + 1) * 128]
                    rhA = w2a[:, 2 * pr:2 * pr + 2, :]
                    rhR = w2r[:, 2 * pr:2 * pr + 2, :]
                    nc.tensor.matmul(pso_t[:], lhA, rhA, start=(jp == 0),
                                     stop=False, perf_mode=DR)
                    nc.tensor.matmul(pso_t[:], lhA, rhR, start=False,
                                     stop=False, perf_mode=DR)
                    nc.tensor.matmul(pso_t[:], lhE, rhA, start=False,
                                     stop=False, perf_mode=DR)

                for tg in range(2):
                    # gelu output in dual fp8: gt8 = fp8(gelu), ge8 the
                    # bf16-vs-fp8 remainder (tri-fp8 fc2 stationaries)
                    gt8 = gtp.tile([128, 32, 512], F8, tag="gt8",
                                   name="gt8")
                    ge8 = gtp.tile([128, 32, 512], F8, tag="ge8",
                                   name="ge8")
                    for j in range(32):
                        ps = psF.tile([128, 512], F32, tag="pf")
                        for i in range(4):
                            wA = wf8_t[:, 2 * i:2 * i + 2,
                                       j * 128:(j + 1) * 128]
                            wC = wfr8_t[:, 2 * i:2 * i + 2,
                                        j * 128:(j + 1) * 128]
                            hA = h2T8[tg][:, 2 * i:2 * i + 2, :]
                            hB = h2Te8[tg][:, 2 * i:2 * i + 2, :]
                            nc.tensor.matmul(ps[:], wA, hA, start=(i == 0),
                                             stop=False, perf_mode=DR)
                            nc.tensor.matmul(ps[:], wC, hA, start=False,
                                             stop=False, perf_mode=DR)
                            nc.tensor.matmul(ps[:], wA, hB, start=False,
                                             stop=(i == 3), perf_mode=DR)
                        gt_b = gtbp.tile([128, 512], BF16, tag="gtb")
                        nc.scalar.activation(gt_b[:], ps[:],
                                             AF.Gelu_apprx_tanh,
                                             bias=bf_t[:, j:j + 1],
                                             scale=1.0 / 512)
                        nc.scalar.activation(gt8[:, j, :], ps[:],
                                             AF.Gelu_apprx_tanh,
                                             bias=bf_t[:, j:j + 1],
                                             scale=1.0 / 512)
                        nc.vector.tensor_tensor(
                            out=ge8[:, j, :], in0=gt_b[:], in1=gt8[:, j, :],
                            op=ALU.subtract)
                    if psO is None:
                        psO = mlp2.enter_context(
                            tc.tile_pool(name="psO", bufs=1, space="PSUM"))
                    for fg in range(2):
                        last = (tg == 1 and fg == 1)
                        pso = [psO.tile([128, 512], F32, tag=f"po{tb}",
                                        name=f"po{tb}") for tb in range(4)]
                        w2s = []
                        for jj in range(4):
                            w2a = wst6.tile([128, 8, 512], F8, tag="wf2a",
                                            name="wf2a")
                            nc.sync.dma_start(
                                w2a[:],
                                WF2[jj * 1024:(jj + 1) * 1024,
                                    fg * 512:(fg + 1) * 512].rearrange(
                                        "(i p2) f -> p2 i f", p2=128))
                            w2r = wst6.tile([128, 8, 512], F8, tag="wf2r",
                                            name="wf2r")
                            nc.sync.dma_start(
                                w2r[:],
                                WF2R[jj * 1024:(jj + 1) * 1024,
                                     fg * 512:(fg + 1) * 512].rearrange(
                                         "(i p2) f -> p2 i f", p2=128))
                            w2s.append((w2a, w2r))
                            if last:
                                continue
                            for pr in range(4):
                                for tb in range(4):
                                    fc2_tri(pso[tb], jj, pr, tb, gt8, ge8,
                                            w2a, w2r)
                        for tb in range(4):
                            if last:
                                # tb-major on the final pass: each output
                                # block drains (bias/add/store) while the
                                # next accumulates, hiding the tail chain
                                for jj in range(4):
                                    for pr in range(4):
                                        fc2_tri(pso[tb], jj, pr, tb, gt8,
                                                ge8, w2s[jj][0], w2s[jj][1])
                            nc.tensor.matmul(
                                pso[tb][:], ones_r[:],
                                bf2_t[0:1, fg * 512:(fg + 1) * 512],
                                start=False, stop=True)
                            gtb = tg * 4 + tb
                            o_t = outp.tile([128, 512], F32, tag="ot")
                            nc.vector.scalar_tensor_tensor(
                                out=o_t[:], in0=pso[tb][:],
                                scalar=1.0 / 128,
                                in1=x2v(gtb)[:, fg * 512:(fg + 1) * 512],
                                op0=ALU.mult, op1=ALU.add)
                            nc.scalar.dma_start(
                                OUT[gtb * 128:(gtb + 1) * 128,
                                    fg * 512:(fg + 1) * 512], o_t[:])


_NC_CACHE = {}


def _get_nc(am_zero=True):
    key = f"nc{int(am_zero)}"
    if key not in _NC_CACHE:
        _NC_CACHE[key] = build_nc(am_zero)
    return _NC_CACHE[key]


def _perm_for(f):
    other = [2 * j + (1 - f) for j in range(8)]
    own = [2 * j + f for j in range(8)]
    blocks = other + own
    return np.concatenate([np.arange(b * 128, (b + 1) * 128) for b in blocks])


def make_in_maps(hidden_states, attention_mask, ln1_g, ln1_b, W_attn, b_attn,
                 W_proj, b_proj, ln2_g, ln2_b, W_fc, b_fc, W_fc2, b_fc2):
    f32 = lambda a: np.asarray(a, dtype=np.float32)
    bf16 = lambda a: np.ascontiguousarray(a.astype(ml_dtypes.bfloat16))
    hidden_states = f32(hidden_states)
    attention_mask = f32(attention_mask)
    ln1_g, ln1_b = f32(ln1_g), f32(ln1_b)
    ln2_g, ln2_b = f32(ln2_g), f32(ln2_b)
    W_attn, b_attn = f32(W_attn), f32(b_attn)
    W_proj, b_proj = f32(W_proj), f32(b_proj)
    W_fc, b_fc = f32(W_fc), f32(b_fc)
    W_fc2, b_fc2 = f32(W_fc2), f32(b_fc2)

    # Fold LN affines into the consuming matmuls (exact algebra, fp64 on host).
    Wa_eff = (ln1_g.astype(np.float64)[:, None] * W_attn).astype(np.float32)
    ba_eff = (b_attn.astype(np.float64)
              + ln1_b.astype(np.float64) @ W_attn).astype(np.float32)
    scale = 1.0 / np.sqrt(np.float32(HD))
    WQn = (Wa_eff[:, 0:D] * scale).astype(np.float32)
    BQn = (ba_eff[0:D] * scale).astype(np.float32)
    WKn, BKn = Wa_eff[:, D:2 * D].copy(), ba_eff[D:2 * D].copy()
    WVn, BVn = Wa_eff[:, 2 * D:3 * D].copy(), ba_eff[2 * D:3 * D].copy()
    Wf_eff = (ln2_g.astype(np.float64)[:, None] * W_fc).astype(np.float32)
    bf_eff = (b_fc.astype(np.float64)
              + ln2_b.astype(np.float64) @ W_fc).astype(np.float32)

    def fp8_pair(w, s):
        """w*s as fp8 plus fp8 remainder (tri-fp8 stationaries)."""
        ws = (w.astype(np.float64) * s).astype(np.float32)
        w8 = ws.astype(ml_dtypes.float8_e4m3)
        wr8 = (ws - w8.astype(np.float32)).astype(ml_dtypes.float8_e4m3)
        assert np.isfinite(w8.astype(np.float32)).all()
        return np.ascontiguousarray(w8), np.ascontiguousarray(wr8)

    WF8, WFR8 = fp8_pair(Wf_eff, 64.0)       # fc1 psum = 8*64 * fc1
    WF28, WF2R8 = fp8_pair(W_fc2, 128.0)     # fc2 psum = 128 * fc2

    shared = {
        "WQ": bf16(WQn),
        "WK": bf16(WKn),
        "WV": bf16(WVn),
        "BQ": np.ascontiguousarray(BQn.reshape(8, 128).T),
        "BK": np.ascontiguousarray(BKn.reshape(8, 128).T),
        "BV": np.ascontiguousarray(BVn[None, :]),
        "WP": bf16(W_proj),
        "WF": WF8,
        "WFR": WFR8,
        "BF": np.ascontiguousarray(bf_eff.reshape(32, 128).T),
        "WF2": WF28,
        "WF2R": WF2R8,
        "BF2": np.ascontiguousarray(b_fc2[None, :] * 128.0),
    }

    am_zero = bool(np.all(attention_mask == 0))
    in_maps, perms = [], []
    for c in range(N_CORES):
        b, f = c >> 1, c & 1
        perm = _perm_for(f)
        perms.append(perm)
        x_ctx = np.ascontiguousarray(hidden_states[b][perm])
        xq = np.ascontiguousarray(hidden_states[b][perm[OWN:]]
                                  + b_proj[None, :])
        gk = perm
        gq = perm[OWN:]
        live = (gk[:, None] <= gq[None, :]).astype(np.float32)
        # 0/1 exp-masks: [:, u*QG:(u+1)*QG] is k-block (base+u) vs q-group 0
        # pair 0: other-parity blocks (0, 1); pair 1: own blocks (8, 9).
        # The relative pattern is g-independent.
        msk = np.zeros((2, 128, 512), np.float32)
        for u, j in enumerate([0, 1]):
            msk[0, :, u * QG:(u + 1) * QG] = live[
                j * 128:(j + 1) * 128, 0:QG]
        for u, j in enumerate([8, 9]):
            msk[1, :, u * QG:(u + 1) * QG] = live[
                j * 128:(j + 1) * 128, 0:QG]
        im = {"X": bf16(x_ctx), "XQ": xq, "MSKE": bf16(msk), **shared}
        if not am_zero:
            am = attention_mask[b, 0, 0, :].astype(np.float64)
            eam = np.exp(am[perm]).astype(np.float32)
            im["EAM"] = np.ascontiguousarray(eam.reshape(SB, 128).T)
        in_maps.append(im)
    return in_maps, perms, am_zero


def kernel(hidden_states, attention_mask, ln1_g, ln1_b, W_attn, b_attn,
           W_proj, b_proj, ln2_g, ln2_b, W_fc, b_fc, W_fc2, b_fc2):
    in_maps, perms, am_zero = make_in_maps(
        hidden_states, attention_mask, ln1_g, ln1_b, W_attn, b_attn,
        W_proj, b_proj, ln2_g, ln2_b, W_fc, b_fc, W_fc2, b_fc2)
    nc = _get_nc(am_zero)
    res = run_bass_kernel_spmd(nc, in_maps, core_ids=list(range(N_CORES)))
    out = np.empty((B, S, D), dtype=np.float32)
    for c in range(N_CORES):
        b = c >> 1
        out[b][perms[c][OWN:]] = res.results[c]["OUT"]
    return out



# revision 41
# speedup vs baseline: 1.0817x; 1.0225x over previous
"""Fused GPT-2 transformer block on 8 Trainium2 NeuronCores.

Sharding: 8 cores = 4 batches x 2 causal-balanced folds. Core (b, f) owns the 8
interleaved 128-token blocks of parity f of batch b (queries), and receives all
2048 tokens of batch b as context, permuted [other-parity blocks | own blocks].
Causality is enforced by exact 0/1 mask multiplies after exp, so a single SPMD
program serves all cores. No collectives.

Layouts: LN1(x) is PE-transposed to hT [D, tok] (bf16); Q/K are produced in
head-major transposed layout (bf16), V in token-major layout with an appended
ones column (so the P@V matmul also accumulates softmax denominators).
Exp runs on the scalar engine in [128,1024] slabs straight from PSUM to bf16;
causal masking is a 0/1 elementwise multiply on the vector engine afterwards
(exp(s+m) == exp(s)*exp(m) with exp(m) in {0,1} exactly). proj/fc matmuls
contract against feature-major lhsT slices. All weights travel as bf16;
LN affines and the proj bias are folded on the host.
"""

import contextlib
import os

import numpy as np
import ml_dtypes

import concourse.bass as bass
import concourse.mybir as mybir
import concourse.tile as tile
from concourse import bacc
from concourse.bass_utils import run_bass_kernel_spmd
from concourse.masks import make_identity

F32 = mybir.dt.float32
F32R = mybir.dt.float32r
BF16 = mybir.dt.bfloat16
F8 = mybir.dt.float8e4
DR = mybir.MatmulPerfMode.DoubleRow
AF = mybir.ActivationFunctionType
ALU = mybir.AluOpType

B, S, D, H = 4, 2048, 1024, 16
HD = D // H          # 64
DFF = 4 * D          # 4096
EPS = 1e-5
MASKED_BIAS = -10000.0
N_CORES = 8

SB = S // 128        # 16 ctx blocks
OWN = S // 2         # 1024 own tokens
OB = OWN // 128      # 8 own blocks
NQG = 4              # q-groups of 256
QG = 256
HSETS = 4            # head sets
HPS = H // HSETS     # 4 heads per set


def _klist(g):
    """ctx k-block indices computed for q-group g (own blocks 2g, 2g+1)."""
    return list(range(0, 2 * g + 2)) + list(range(8, 8 + 2 * g + 2))


def build_nc(am_zero=True):
    nc = bacc.Bacc("TRN2", target_bir_lowering=False, debug=False,
                   num_devices=N_CORES)

    X = nc.dram_tensor("X", [S, D], BF16, kind="ExternalInput")
    XQ = nc.dram_tensor("XQ", [OWN, D], F32, kind="ExternalInput")
    MSKE = nc.dram_tensor("MSKE", [2, 128, 512], BF16, kind="ExternalInput")
    EAM = (None if am_zero else
           nc.dram_tensor("EAM", [128, SB], F32, kind="ExternalInput"))
    WQ = (nc.dram_tensor("WQ", [D, D], F8, kind="ExternalInput"),
          nc.dram_tensor("WQR", [D, D], F8, kind="ExternalInput"))
    WK = (nc.dram_tensor("WK", [D, D], F8, kind="ExternalInput"),
          nc.dram_tensor("WKR", [D, D], F8, kind="ExternalInput"))
    WV = (nc.dram_tensor("WV", [D, D], F8, kind="ExternalInput"),
          nc.dram_tensor("WVR", [D, D], F8, kind="ExternalInput"))
    BQ = nc.dram_tensor("BQ", [128, 8], F32, kind="ExternalInput")
    WP = (nc.dram_tensor("WP", [D, D], F8, kind="ExternalInput"),
          nc.dram_tensor("WPR", [D, D], F8, kind="ExternalInput"))
    WF = nc.dram_tensor("WF", [D, DFF], F8, kind="ExternalInput")
    WFR = nc.dram_tensor("WFR", [D, DFF], F8, kind="ExternalInput")
    BF = nc.dram_tensor("BF", [128, 32], F32, kind="ExternalInput")
    WF2 = nc.dram_tensor("WF2", [DFF, D], F8, kind="ExternalInput")
    WF2R = nc.dram_tensor("WF2R", [DFF, D], F8, kind="ExternalInput")
    BF2 = nc.dram_tensor("BF2", [1, D], F32, kind="ExternalInput")
    OUT = nc.dram_tensor("OUT", [OWN, D], F32, kind="ExternalOutput")

    with tile.TileContext(nc) as tc:
        _body(nc, tc, X, XQ, MSKE, EAM, WQ, WK, WV, BQ, WP, WF, WFR,
              BF, WF2, WF2R, BF2, OUT, am_zero)
    nc.compile()
    return nc


def _ln_stats(nc, stat, src, eps_t):
    """LN stats of src [128, D] -> (rinv [128,1], nb [128,1]) with
    nb = -mean * rinv."""
    sub = 512
    nsub = D // sub
    xs = src.rearrange("p (n s) -> p n s", s=sub)
    stats = stat.tile([128, nsub, nc.vector.BN_STATS_DIM], F32, tag="bnst")
    for j in range(nsub):
        nc.vector.bn_stats(out=stats[:, j, :], in_=xs[:, j, :])
    mv = stat.tile([128, nc.vector.BN_AGGR_DIM], F32, tag="bnag")
    nc.vector.bn_aggr(out=mv[:, :], in_=stats[:, :, :])
    nc.scalar.activation(out=mv[:, 1:2], in_=mv[:, 1:2], func=AF.Sqrt,
                         bias=eps_t[:], scale=1.0)
    rinv = stat.tile([128, 1], F32, tag="rinv")
    nc.vector.reciprocal(out=rinv[:], in_=mv[:, 1:2])
    nb = stat.tile([128, 1], F32, tag="nb")
    nc.vector.scalar_tensor_tensor(out=nb[:], in0=mv[:, 0:1], scalar=-1.0,
                                   in1=rinv[:], op0=ALU.mult, op1=ALU.mult)
    return rinv, nb


def _body(nc, tc, X, XQ, MSKE, EAM, WQ, WK, WV, BQ, WP, WF, WFR,
          BF, WF2, WF2R, BF2, OUT, am_zero=True):
    PL = int(os.environ.get("KPHASES", "9"))
    with contextlib.ExitStack() as top:
        cst = top.enter_context(tc.tile_pool(name="cst", bufs=1))
        stat = top.enter_context(tc.tile_pool(name="stat", bufs=4))

        ident = cst.tile([128, 128], F32)
        make_identity(nc, ident[:])
        ones_f = cst.tile([1, 128], F32)
        nc.vector.memset(ones_f[:], 1.0)
        # V's appended denominator column is 512 so the P@V matmul's
        # denominator absorbs the 1/512 psum scale of vS (ratio is exact)
        ones_c3 = cst.tile([128, HPS, 1], BF16)
        nc.vector.memset(ones_c3[:], 512.0)
        ones_r = cst.tile([1, 128], F32R)   # bias-row lhsT
        nc.scalar.copy(ones_r[:], ones_f[:])
        eps_t = cst.tile([128, 1], F32)
        nc.vector.memset(eps_t[:], EPS)
        nlog8_t = cst.tile([128, 1], F32)   # -ln(8): exp-shift bias
        nc.vector.memset(nlog8_t[:], -2.0794415416798357)
        ident_b = cst.tile([128, 128], BF16)
        nc.scalar.copy(ident_b[:], ident[:])

        with contextlib.ExitStack() as attn_stack:
            atp = attn_stack.enter_context(tc.tile_pool(name="atp", bufs=1))
            aT = [atp.tile([128, OWN], BF16, tag=f"aT{p}", name=f"aT{p}")
                  for p in range(8)]

            with contextlib.ExitStack() as ht_stack:
                ht = ht_stack.enter_context(tc.tile_pool(name="ht", bufs=1))
                # hT8[tg] : [128, db, 512] fp8 ~ 8*LN1(x).T plus remainder
                # hTe8 (tri-fp8 DoubleRow moving operands, db-pairs adjacent)
                hT8 = [ht.tile([128, 8, 512], F8, tag=f"hT8_{tg}",
                               name=f"hT8_{tg}") for tg in range(4)]
                hTe8 = [ht.tile([128, 8, 512], F8, tag=f"hTe8_{tg}",
                                name=f"hTe8_{tg}") for tg in range(4)]

                with contextlib.ExitStack() as hs_stack:
                    # attention-phase pools are created BEFORE the phase-1
                    # pools so phase-1 buffer teardown never aliases them
                    kvq = hs_stack.enter_context(
                        tc.tile_pool(name="kvq", bufs=2))
                    mskp = hs_stack.enter_context(
                        tc.tile_pool(name="mskp", bufs=1))
                    att = hs_stack.enter_context(
                        tc.tile_pool(name="att", bufs=5))
                    wst = hs_stack.enter_context(
                        tc.tile_pool(name="wstA", bufs=2))
                    psKV = hs_stack.enter_context(
                        tc.tile_pool(name="psKV", bufs=2, space="PSUM"))
                    psS = hs_stack.enter_context(
                        tc.tile_pool(name="psS", bufs=2, space="PSUM"))

                    # first X chunk goes to the head of the DMA queue so
                    # LN1 starts as early as possible
                    x_first = mskp.tile([128, 2, D], BF16, tag="xf",
                                        name="xf")
                    nc.sync.dma_start(
                        x_first[:],
                        X[0:256, :].rearrange("(i p) d -> p i d", p=128))

                    # 0/1 exp-masks (bf16) + per-token exp(attn-mask)
                    mskE = mskp.tile([128, 2, 512], BF16, tag="mskE",
                                     name="mskE")
                    nc.sync.dma_start(mskE[:],
                                      MSKE[:, :, :].rearrange("m p f -> p m f"))
                    eam = None
                    if not am_zero:
                        eam = mskp.tile([128, SB], F32, tag="eam", name="eam")
                        nc.sync.dma_start(eam[:], EAM[:, :])
                    bq_t = mskp.tile([128, 8], F32, tag="bq", name="bq")
                    nc.sync.dma_start(bq_t[:], BQ[:, :])

                    # ---- Phase 1: LN1 over ctx + transpose -> hT ----
                    # 4 token-blocks transpose into one PSUM bank, so each
                    # hT[db][tg] tile is produced by a single wide copy.
                    with tc.tile_pool(name="psT", bufs=2, space="PSUM") \
                            as psT, \
                         tc.tile_pool(name="xin1", bufs=4) as xin, \
                         tc.tile_pool(name="xln", bufs=6) as xlnp:
                        for xg in range(4):
                            xts = []
                            for xh in range(2):
                                if xg == 0 and xh == 0:
                                    xts.append(x_first)
                                    continue
                                x_t = xin.tile([128, 2, D], BF16, tag="x1")
                                nc.sync.dma_start(
                                    x_t[:],
                                    X[xg * 512 + xh * 256:
                                      xg * 512 + (xh + 1) * 256,
                                      :].rearrange("(i p) d -> p i d",
                                                   p=128))
                                xts.append(x_t)
                            xls = []
                            for i in range(4):
                                xv = xts[i // 2][:, i % 2, :]
                                rinv, nb = _ln_stats(nc, stat, xv, eps_t)
                                x_ln = xlnp.tile([128, D], BF16, tag="xln")
                                nc.scalar.activation(out=x_ln[:], in_=xv,
                                                     func=AF.Identity,
                                                     bias=nb[:],
                                                     scale=rinv[:])
                                xls.append(x_ln)
                            for db in range(8):
                                pt = psT.tile([128, 512], BF16, tag="tp")
                                for i in range(4):
                                    nc.tensor.transpose(
                                        pt[:, i * 128:(i + 1) * 128],
                                        xls[i][:, db * 128:(db + 1) * 128],
                                        ident_b[:])
                                nc.scalar.activation(
                                    out=hT8[xg][:, db, :], in_=pt[:],
                                    func=AF.Copy, scale=8.0)
                                nc.vector.scalar_tensor_tensor(
                                    out=hTe8[xg][:, db, :], in0=pt[:],
                                    scalar=8.0, in1=hT8[xg][:, db, :],
                                    op0=ALU.mult, op1=ALU.subtract)

                    if PL < 2:
                        return
                    # psA reuses psT's freed banks; the region-reuse
                    # dependency (first pa write after last phase-1
                    # transpose copy) is subsumed by the data dependency
                    # attention -> K/V -> hT -> phase 1.
                    psA = hs_stack.enter_context(
                        tc.tile_pool(name="psA", bufs=2, space="PSUM"))
                    def tri(ps_t, wa, wr, tg, i, first, last):
                        lhA = wa[:, 2 * i:2 * i + 2, :]
                        lhR = wr[:, 2 * i:2 * i + 2, :]
                        rhA = hT8[tg][:, 2 * i:2 * i + 2, :]
                        rhE = hTe8[tg][:, 2 * i:2 * i + 2, :]
                        nc.tensor.matmul(ps_t, lhA, rhA, start=first,
                                         stop=False, perf_mode=DR)
                        nc.tensor.matmul(ps_t, lhR, rhA, start=False,
                                         stop=False, perf_mode=DR)
                        nc.tensor.matmul(ps_t, lhA, rhE, start=False,
                                         stop=last, perf_mode=DR)

                    def alloc_kvq():
                        kT = [kvq.tile([128, S], BF16, tag=f"kT{p}",
                                       name=f"kT{p}") for p in range(2)]
                        qT = [kvq.tile([128, OWN], BF16, tag=f"qT{p}",
                                       name=f"qT{p}") for p in range(2)]
                        vS = [kvq.tile([128, HPS, HD + 1], BF16,
                                       tag=f"vS{tb}", name=f"vS{tb}")
                              for tb in range(SB)]
                        return kT, qT, vS

                    def emit_wdma(hs):
                        wkq = []
                        for p in range(2):
                            fcol = hs * 256 + p * 128
                            wk_p, wq_p = [], []
                            for wt, dr in (("a", 0), ("r", 1)):
                                wk_t = wst.tile([128, 8, 128], F8,
                                                tag=f"wk{p}{wt}",
                                                name=f"wk{p}{wt}")
                                nc.sync.dma_start(
                                    wk_t[:],
                                    WK[dr][:, fcol:fcol + 128].rearrange(
                                        "(i p2) f -> p2 i f", p2=128))
                                wk_p.append(wk_t)
                                wq_t = wst.tile([128, 8, 128], F8,
                                                tag=f"wq{p}{wt}",
                                                name=f"wq{p}{wt}")
                                nc.sync.dma_start(
                                    wq_t[:],
                                    WQ[dr][:, fcol:fcol + 128].rearrange(
                                        "(i p2) f -> p2 i f", p2=128))
                                wq_p.append(wq_t)
                            wkq.append((wk_p, wq_p))
                        wv_p = []
                        for wt, dr in (("a", 0), ("r", 1)):
                            wv_t = wst.tile([128, 8, 256], F8,
                                            tag=f"wv{wt}", name=f"wv{wt}")
                            nc.sync.dma_start(
                                wv_t[:],
                                WV[dr][:, hs * 256:(hs + 1) * 256].rearrange(
                                    "(i p2) f -> p2 i f", p2=128))
                            wv_p.append(wv_t)
                        return wkq, wv_p

                    def kvq_groups(hs, tiles, weights):
                        """One psum-group emitter per yield; interleaved
                        into the previous head set's attention so the PE
                        has ready work while it waits on exp."""
                        kT, qT, vS = tiles
                        wkq, wv_p = weights
                        for p in range(2):
                            (wk8, wkr8), (wq8, wqr8) = wkq[p]
                            bcol = hs * 2 + p
                            for tg in range(4):
                                def k_grp(p=p, tg=tg, wk8=wk8, wkr8=wkr8):
                                    ps = psKV.tile([128, 512], F32,
                                                   tag="pk")
                                    for i in range(4):
                                        tri(ps[:], wk8, wkr8, tg, i,
                                            i == 0, i == 3)
                                    dst = kT[p][:, tg * 512:(tg + 1) * 512]
                                    if tg % 2 == 0:
                                        nc.vector.tensor_copy(dst, ps[:])
                                    else:
                                        nc.scalar.copy(dst, ps[:])
                                yield k_grp
                            for tg in range(2):
                                def q_grp(p=p, tg=tg, bcol=bcol, wq8=wq8,
                                          wqr8=wqr8):
                                    ps = psKV.tile([128, 512], F32,
                                                   tag="pk")
                                    for i in range(4):
                                        tri(ps[:], wq8, wqr8, 2 + tg, i,
                                            i == 0, i == 3)
                                    nc.vector.tensor_scalar_add(
                                        out=qT[p][:,
                                                  tg * 512:(tg + 1) * 512],
                                        in0=ps[:],
                                        scalar1=bq_t[:, bcol:bcol + 1])
                                yield q_grp
                        for tb in range(SB):
                            def v_grp(tb=tb):
                                ps = psKV.tile([128, 512], F32, tag="pk")
                                tgv, c0 = tb // 4, (tb % 4) * 128
                                for i in range(4):
                                    lhA = hT8[tgv][:, 2 * i:2 * i + 2,
                                                   c0:c0 + 128]
                                    lhE = hTe8[tgv][:, 2 * i:2 * i + 2,
                                                    c0:c0 + 128]
                                    nc.tensor.matmul(
                                        ps[:, 0:256], lhA,
                                        wv_p[0][:, 2 * i:2 * i + 2, :],
                                        start=(i == 0), stop=False,
                                        perf_mode=DR)
                                    nc.tensor.matmul(
                                        ps[:, 0:256], lhA,
                                        wv_p[1][:, 2 * i:2 * i + 2, :],
                                        start=False, stop=False,
                                        perf_mode=DR)
                                    nc.tensor.matmul(
                                        ps[:, 0:256], lhE,
                                        wv_p[0][:, 2 * i:2 * i + 2, :],
                                        start=False, stop=(i == 3),
                                        perf_mode=DR)
                                nc.vector.tensor_copy(
                                    vS[tb][:, :, 0:HD],
                                    ps[:, 0:256].rearrange(
                                        "p (h d) -> p h d", d=HD))
                                nc.gpsimd.tensor_copy(
                                    vS[tb][:, :, HD:HD + 1], ones_c3[:])
                            yield v_grp

                    cur_tiles = alloc_kvq()
                    cur_w = emit_wdma(0)
                    for grp in kvq_groups(0, cur_tiles, cur_w):
                        grp()
                    for hs in range(HSETS):
                        kT, qT, vS = cur_tiles
                        if hs + 1 < HSETS:
                            nxt_tiles = alloc_kvq()
                            nxt_w = emit_wdma(hs + 1)
                            pending = kvq_groups(hs + 1, nxt_tiles, nxt_w)
                        else:
                            nxt_tiles = nxt_w = None
                            pending = iter(())

                        # ---- attention for this head set ----
                        # PV runs as P.T @ V: the exp block [128k, 128q] is
                        # the stationary operand and V [128k, 65] the bf16
                        # moving operand (65 rows/matmul). The ones column
                        # of V accumulates softmax denominators into the
                        # output's col 64, so normalization is a cheap
                        # per-partition scalar multiply; a PE transpose
                        # brings the normalized [q, feat] block back to
                        # feature-major aT for proj.
                        for g in range(NQG):
                            kl = _klist(g)
                            nquads = g + 1
                            for p in range(2):
                                # one bank per (g, p): accumulators for the
                                # two 64-row head-subs at cols 0:65/128:193
                                # (sub 0) and 256:321/384:449 (sub 1); the
                                # normalized output is then PE-transposed
                                # back over cols 0:128 (after the
                                # accumulators are read) so ONE [128,256]
                                # copy drains both subs into aT.
                                pab = psA.tile([128, 2 * QG], F32, tag="pab")
                                pq = [[pab[:, 0:HD + 1],
                                       pab[:, 128:128 + HD + 1]],
                                      [pab[:, 256:256 + HD + 1],
                                       pab[:, 384:384 + HD + 1]]]
                                for sub in range(2):
                                    h = 2 * p + sub
                                    for qd in range(nquads):
                                        blocks = kl[4 * qd:4 * qd + 4]
                                        pss = psS.tile([128, 1024], F32,
                                                       tag="ps")
                                        for u in range(4):
                                            kb = blocks[u]
                                            nc.tensor.matmul(
                                                pss[:, u * QG:(u + 1) * QG],
                                                kT[p][sub * 64:
                                                      (sub + 1) * 64,
                                                      kb * 128:
                                                      (kb + 1) * 128],
                                                qT[p][sub * 64:
                                                      (sub + 1) * 64,
                                                      g * QG:(g + 1) * QG],
                                                start=True, stop=True)
                                        # next head set's K/V/Q: ready PE
                                        # work while exp runs on Act
                                        grp = next(pending, None)
                                        if grp is not None:
                                            grp()
                                        wide = att.tile([128, 1024], BF16,
                                                        tag="wide",
                                                        name="wide")
                                        # psum holds 2^21 * logits (512*k x
                                        # 4096*q); the -ln(8) bias keeps exp
                                        # outputs small (softmax-invariant)
                                        nc.scalar.activation(
                                            wide[:], pss[:], AF.Exp,
                                            bias=nlog8_t[:],
                                            scale=2.0 ** -21)
                                        if qd == g // 2:
                                            sl = wide[:, (g % 2) * 512:
                                                      (g % 2) * 512 + 512]
                                            nc.vector.tensor_mul(
                                                sl, sl, mskE[:, 0, :])
                                        if qd == g:
                                            sl = wide[:, 512:1024]
                                            nc.vector.tensor_mul(
                                                sl, sl, mskE[:, 1, :])
                                        if not am_zero:
                                            for u in range(4):
                                                kb = blocks[u]
                                                sl = wide[:, u * QG:
                                                          (u + 1) * QG]
                                                nc.vector.tensor_scalar_mul(
                                                    out=sl, in0=sl,
                                                    scalar1=eam[:,
                                                                kb:kb + 1])
                                        # one accumulation group for the
                                        # whole bank: start clears the
                                        # bank-wide has_written bits, so
                                        # only the very first matmul may
                                        # carry it
                                        for u in range(4):
                                            kb = blocks[u]
                                            for qs in range(2):
                                                nc.tensor.matmul(
                                                    pq[sub][qs],
                                                    wide[:,
                                                         u * QG + qs * 128:
                                                         u * QG + qs * 128
                                                         + 128],
                                                    vS[kb][:, h, :],
                                                    start=(sub == 0
                                                           and qd == 0
                                                           and u == 0
                                                           and qs == 0),
                                                    stop=(sub == 1
                                                          and qd ==
                                                          nquads - 1
                                                          and u == 3
                                                          and qs == 1),
                                                    skip_group_check=True)
                                ap_idx = 2 * hs + p
                                anrms = []
                                for sub in range(2):
                                    for qs in range(2):
                                        rec = att.tile([128, 1], F32,
                                                       tag="rec")
                                        nc.vector.reciprocal(
                                            rec[:],
                                            pq[sub][qs][:, HD:HD + 1])
                                        anrm = att.tile([128, HD], BF16,
                                                        tag="anrm")
                                        nc.vector.tensor_scalar_mul(
                                            out=anrm[:],
                                            in0=pq[sub][qs][:, 0:HD],
                                            scalar1=rec[:])
                                        anrms.append((sub, qs, anrm))
                                for sub, qs, anrm in anrms:
                                    nc.tensor.transpose(
                                        pab[sub * 64:sub * 64 + HD,
                                            qs * 64:(qs + 1) * 64].bitcast(
                                                BF16),
                                        anrm[:], ident_b[:])
                                dst = aT[ap_idx][:, g * QG:(g + 1) * QG]
                                src = pab[:, 0:128].bitcast(BF16)
                                if p == 0:
                                    nc.vector.tensor_copy(dst, src)
                                else:
                                    nc.scalar.copy(dst, src)
                        for grp in pending:
                            grp()
                        cur_tiles, cur_w = nxt_tiles, nxt_w

            if PL < 4:
                return
            # ---- proj + residual -> x2 ; prefetch WF/BF/BF2 ----
            psT2 = top.enter_context(
                tc.tile_pool(name="psT2", bufs=2, space="PSUM"))
            psF = top.enter_context(
                tc.tile_pool(name="psF", bufs=2, space="PSUM"))
            x2p = top.enter_context(tc.tile_pool(name="x2p", bufs=1,
                                                 side="right"))
            wfp = top.enter_context(tc.tile_pool(name="wfp", bufs=1,
                                                 side="right"))

            # proj weights + residual inputs issue FIRST; the long WF
            # prefetch queues behind them on the SP queue
            wstp = attn_stack.enter_context(tc.tile_pool(name="wstP",
                                                         bufs=1))
            wpt = []
            for fg in range(2):
                pair = []
                for wt, dr in (("a", 0), ("r", 1)):
                    w_t = wstp.tile([128, 8, 512], F8, tag=f"wp{fg}{wt}",
                                    name=f"wp{fg}{wt}")
                    nc.sync.dma_start(
                        w_t[:],
                        WP[dr][:, fg * 512:(fg + 1) * 512].rearrange(
                            "(i p2) f -> p2 i f", p2=128))
                    pair.append(w_t)
                wpt.append(pair)
            x2 = [x2p.tile([128, 4, D], F32, tag=f"x2{i}", name=f"x2{i}")
                  for i in range(2)]
            for i in range(2):
                nc.sync.dma_start(
                    x2[i][:],
                    XQ[i * 512:(i + 1) * 512, :].rearrange(
                        "(i2 p) d -> p i2 d", p=128))

            wf8_t = wfp.tile([128, 8, DFF], F8, tag="wf8", name="wf8")
            wfr8_t = wfp.tile([128, 8, DFF], F8, tag="wfr8", name="wfr8")
            for db in range(8):
                nc.sync.dma_start(wf8_t[:, db, :],
                                  WF[db * 128:(db + 1) * 128, :])
                nc.sync.dma_start(wfr8_t[:, db, :],
                                  WFR[db * 128:(db + 1) * 128, :])
            bf_t = wfp.tile([128, 32], F32, tag="bf", name="bf")
            nc.sync.dma_start(bf_t[:], BF[:, :])
            bf2_t = wfp.tile([1, D], F32R, tag="bf2", name="bf2")
            nc.sync.dma_start(bf2_t[:], BF2[:, :].bitcast(F32R))

            def x2v(tb):
                return x2[tb // 4][:, tb % 4, :]

            # bulk-convert attention output to dual fp8 (8*a + remainder)
            aT8 = atp.tile([128, 8, OWN], F8, tag="aT8", name="aT8")
            aTe8 = atp.tile([128, 8, OWN], F8, tag="aTe8", name="aTe8")
            for ab in range(8):
                nc.scalar.activation(out=aT8[:, ab, :], in_=aT[ab][:],
                                     func=AF.Copy, scale=8.0)
                nc.vector.scalar_tensor_tensor(
                    out=aTe8[:, ab, :], in0=aT[ab][:], scalar=8.0,
                    in1=aT8[:, ab, :], op0=ALU.mult, op1=ALU.subtract)

            with tc.tile_pool(name="psP", bufs=2, space="PSUM") as psP:
                for tb in range(OB):
                    for fg in range(2):
                        ps = psP.tile([128, 512], F32, tag="pp")
                        for i in range(4):
                            lhA = aT8[:, 2 * i:2 * i + 2,
                                      tb * 128:(tb + 1) * 128]
                            lhE = aTe8[:, 2 * i:2 * i + 2,
                                       tb * 128:(tb + 1) * 128]
                            rhA = wpt[fg][0][:, 2 * i:2 * i + 2, :]
                            rhR = wpt[fg][1][:, 2 * i:2 * i + 2, :]
                            nc.tensor.matmul(ps[:], lhA, rhA,
                                             start=(i == 0), stop=False,
                                             perf_mode=DR)
                            nc.tensor.matmul(ps[:], lhA, rhR, start=False,
                                             stop=False, perf_mode=DR)
                            nc.tensor.matmul(ps[:], lhE, rhA, start=False,
                                             stop=(i == 3), perf_mode=DR)
                        dst = x2v(tb)[:, fg * 512:(fg + 1) * 512]
                        nc.vector.scalar_tensor_tensor(
                            out=dst, in0=ps[:], scalar=2.0 ** -9,
                            in1=dst, op0=ALU.mult, op1=ALU.add)

        if PL < 5:
            return
        # ---- LN2 + transpose -> h2T8 (+ fp8 residual) ; then MLP ----
        with contextlib.ExitStack() as mlp_stack:
            ht2 = mlp_stack.enter_context(tc.tile_pool(name="ht2", bufs=1))
            # fp8 dual representation of LN2(x2).T: h2T8 ~ 8*h2, h2Te8 the
            # quantization remainder (tri-fp8 DoubleRow matmul operands)
            h2T8 = [ht2.tile([128, 8, 512], F8, tag=f"h2T8_{tg}",
                             name=f"h2T8_{tg}") for tg in range(2)]
            h2Te8 = [ht2.tile([128, 8, 512], F8, tag=f"h2Te8_{tg}",
                              name=f"h2Te8_{tg}") for tg in range(2)]
            with tc.tile_pool(name="xln2", bufs=6) as xlnp:
                for tg in range(2):
                    xls = []
                    for i in range(4):
                        tb = tg * 4 + i
                        rinv, nb = _ln_stats(nc, stat, x2v(tb), eps_t)
                        x_ln = xlnp.tile([128, D], BF16, tag="xln")
                        nc.scalar.activation(out=x_ln[:], in_=x2v(tb),
                                             func=AF.Identity,
                                             bias=nb[:], scale=rinv[:])
                        xls.append(x_ln)
                    for db in range(8):
                        pt = psT2.tile([128, 512], BF16, tag="tp")
                        for i in range(4):
                            nc.tensor.transpose(
                                pt[:, i * 128:(i + 1) * 128],
                                xls[i][:, db * 128:(db + 1) * 128],
                                ident_b[:])
                        nc.scalar.activation(
                            out=h2T8[tg][:, db, :], in_=pt[:],
                            func=AF.Copy, scale=8.0)
                        nc.vector.scalar_tensor_tensor(
                            out=h2Te8[tg][:, db, :], in0=pt[:], scalar=8.0,
                            in1=h2T8[tg][:, db, :], op0=ALU.mult,
                            op1=ALU.subtract)

            if PL < 6:
                return
            with contextlib.ExitStack() as mlp2:
                gtp = mlp2.enter_context(tc.tile_pool(name="gtp", bufs=1))
                gtbp = mlp2.enter_context(tc.tile_pool(name="gtb", bufs=4))
                wst6 = mlp2.enter_context(tc.tile_pool(name="wstF6", bufs=4))
                outp = mlp2.enter_context(tc.tile_pool(name="outp", bufs=3))
                psO = None

                def fc2_tri(pso_t, jj, pr, tb, gt8, ge8, w2a, w2r):
                    jp = jj * 4 + pr
                    j0 = 2 * jp
                    lhA = gt8[:, j0:j0 + 2, tb * 128:(tb + 1) * 128]
                    lhE = ge8[:, j0:j0 + 2, tb * 128:(tb + 1) * 128]
                    rhA = w2a[:, 2 * pr:2 * pr + 2, :]
                    rhR = w2r[:, 2 * pr:2 * pr + 2, :]
                    nc.tensor.matmul(pso_t[:], lhA, rhA, start=(jp == 0),
                                     stop=False, perf_mode=DR)
                    nc.tensor.matmul(pso_t[:], lhA, rhR, start=False,
                                     stop=False, perf_mode=DR)
                    nc.tensor.matmul(pso_t[:], lhE, rhA, start=False,
                                     stop=False, perf_mode=DR)

                for tg in range(2):
                    # gelu output in dual fp8: gt8 = fp8(gelu), ge8 the
                    # bf16-vs-fp8 remainder (tri-fp8 fc2 stationaries)
                    gt8 = gtp.tile([128, 32, 512], F8, tag="gt8",
                                   name="gt8")
                    ge8 = gtp.tile([128, 32, 512], F8, tag="ge8",
                                   name="ge8")
                    for j in range(32):
                        ps = psF.tile([128, 512], F32, tag="pf")
                        for i in range(4):
                            wA = wf8_t[:, 2 * i:2 * i + 2,
                                       j * 128:(j + 1) * 128]
                            wC = wfr8_t[:, 2 * i:2 * i + 2,
                                        j * 128:(j + 1) * 128]
                            hA = h2T8[tg][:, 2 * i:2 * i + 2, :]
                            hB = h2Te8[tg][:, 2 * i:2 * i + 2, :]
                            nc.tensor.matmul(ps[:], wA, hA, start=(i == 0),
                                             stop=False, perf_mode=DR)
                            nc.tensor.matmul(ps[:], wC, hA, start=False,
                                             stop=False, perf_mode=DR)
                            nc.tensor.matmul(ps[:], wA, hB, start=False,
                                             stop=(i == 3), perf_mode=DR)
                        gt_b = gtbp.tile([128, 512], BF16, tag="gtb")
                        nc.scalar.activation(gt_b[:], ps[:],
                                             AF.Gelu_apprx_tanh,
                                             bias=bf_t[:, j:j + 1],
                                             scale=1.0 / 512)
                        nc.vector.tensor_copy(gt8[:, j, :], gt_b[:])
                        nc.gpsimd.tensor_tensor(
                            out=ge8[:, j, :], in0=gt_b[:], in1=gt8[:, j, :],
                            op=ALU.subtract)
                    if psO is None:
                        psO = mlp2.enter_context(
                            tc.tile_pool(name="psO", bufs=1, space="PSUM"))
                    for fg in range(2):
                        last = (tg == 1 and fg == 1)
                        pso = [psO.tile([128, 512], F32, tag=f"po{tb}",
                                        name=f"po{tb}") for tb in range(4)]
                        w2s = []
                        for jj in range(4):
                            w2a = wst6.tile([128, 8, 512], F8, tag="wf2a",
                                            name="wf2a")
                            nc.sync.dma_start(
                                w2a[:],
                                WF2[jj * 1024:(jj + 1) * 1024,
                                    fg * 512:(fg + 1) * 512].rearrange(
                                        "(i p2) f -> p2 i f", p2=128))
                            w2r = wst6.tile([128, 8, 512], F8, tag="wf2r",
                                            name="wf2r")
                            nc.sync.dma_start(
                                w2r[:],
                                WF2R[jj * 1024:(jj + 1) * 1024,
                                     fg * 512:(fg + 1) * 512].rearrange(
                                         "(i p2) f -> p2 i f", p2=128))
                            w2s.append((w2a, w2r))
                            if last:
                                continue
                            for pr in range(4):
                                for tb in range(4):
                                    fc2_tri(pso[tb], jj, pr, tb, gt8, ge8,
                                            w2a, w2r)
                        for tb in range(4):
                            if last:
                                # tb-major on the final pass: each output
                                # block drains (bias/add/store) while the
                                # next accumulates, hiding the tail chain
                                for jj in range(4):
                                    for pr in range(4):
                                        fc2_tri(pso[tb], jj, pr, tb, gt8,
                                                ge8, w2s[jj][0], w2s[jj][1])
                            nc.tensor.matmul(
                                pso[tb][:], ones_r[:],
                                bf2_t[0:1, fg * 512:(fg + 1) * 512],
                                start=False, stop=True)
                            gtb = tg * 4 + tb
                            o_t = outp.tile([128, 512], F32, tag="ot")
                            nc.vector.scalar_tensor_tensor(
                                out=o_t[:], in0=pso[tb][:],
                                scalar=1.0 / 128,
                                in1=x2v(gtb)[:, fg * 512:(fg + 1) * 512],
                                op0=ALU.mult, op1=ALU.add)
                            nc.scalar.dma_start(
                                OUT[gtb * 128:(gtb + 1) * 128,
                                    fg * 512:(fg + 1) * 512], o_t[:])


_NC_CACHE = {}


def _get_nc(am_zero=True):
    key = f"nc{int(am_zero)}"
    if key not in _NC_CACHE:
        _NC_CACHE[key] = build_nc(am_zero)
    return _NC_CACHE[key]


def _perm_for(f):
    other = [2 * j + (1 - f) for j in range(8)]
    own = [2 * j + f for j in range(8)]
    blocks = other + own
    return np.concatenate([np.arange(b * 128, (b + 1) * 128) for b in blocks])


def make_in_maps(hidden_states, attention_mask, ln1_g, ln1_b, W_attn, b_attn,
                 W_proj, b_proj, ln2_g, ln2_b, W_fc, b_fc, W_fc2, b_fc2):
    f32 = lambda a: np.asarray(a, dtype=np.float32)
    bf16 = lambda a: np.ascontiguousarray(a.astype(ml_dtypes.bfloat16))
    hidden_states = f32(hidden_states)
    attention_mask = f32(attention_mask)
    ln1_g, ln1_b = f32(ln1_g), f32(ln1_b)
    ln2_g, ln2_b = f32(ln2_g), f32(ln2_b)
    W_attn, b_attn = f32(W_attn), f32(b_attn)
    W_proj, b_proj = f32(W_proj), f32(b_proj)
    W_fc, b_fc = f32(W_fc), f32(b_fc)
    W_fc2, b_fc2 = f32(W_fc2), f32(b_fc2)

    # Fold LN affines into the consuming matmuls (exact algebra, fp64 on host).
    Wa_eff = (ln1_g.astype(np.float64)[:, None] * W_attn).astype(np.float32)
    ba_eff = (b_attn.astype(np.float64)
              + ln1_b.astype(np.float64) @ W_attn).astype(np.float32)
    scale = 1.0 / np.sqrt(np.float32(HD))
    WQn = (Wa_eff[:, 0:D] * scale).astype(np.float32)
    BQn = (ba_eff[0:D] * scale).astype(np.float32)
    WKn, BKn = Wa_eff[:, D:2 * D].copy(), ba_eff[D:2 * D].copy()
    WVn, BVn = Wa_eff[:, 2 * D:3 * D].copy(), ba_eff[2 * D:3 * D].copy()
    Wf_eff = (ln2_g.astype(np.float64)[:, None] * W_fc).astype(np.float32)
    bf_eff = (b_fc.astype(np.float64)
              + ln2_b.astype(np.float64) @ W_fc).astype(np.float32)

    def fp8_pair(w, s):
        """w*s as fp8 plus fp8 remainder (tri-fp8 stationaries)."""
        ws = (w.astype(np.float64) * s).astype(np.float32)
        w8 = ws.astype(ml_dtypes.float8_e4m3)
        wr8 = (ws - w8.astype(np.float32)).astype(ml_dtypes.float8_e4m3)
        assert np.isfinite(w8.astype(np.float32)).all()
        return np.ascontiguousarray(w8), np.ascontiguousarray(wr8)

    WF8, WFR8 = fp8_pair(Wf_eff, 64.0)       # fc1 psum = 8*64 * fc1
    WF28, WF2R8 = fp8_pair(W_fc2, 128.0)     # fc2 psum = 128 * fc2
    WQ8, WQR8 = fp8_pair(WQn, 512.0)         # q psum = 8*512 * q_eff
    WK8, WKR8 = fp8_pair(WKn, 64.0)          # k psum = 8*64 * k
    WV8, WVR8 = fp8_pair(WVn, 64.0)          # v psum = 8*64 * v
    WP8, WPR8 = fp8_pair(W_proj, 64.0)       # proj psum = 8*64 * proj

    shared = {
        "WQ": WQ8, "WQR": WQR8,
        "WK": WK8, "WKR": WKR8,
        "WV": WV8, "WVR": WVR8,
        # q bias at the q-psum scale (folded during psum->qT copy); the k
        # bias is dropped exactly (softmax is invariant to per-query
        # constants), the v bias is folded into the proj residual below
        "BQ": np.ascontiguousarray(BQn.reshape(8, 128).T * 4096.0),
        "WP": WP8, "WPR": WPR8,
        "WF": WF8,
        "WFR": WFR8,
        "BF": np.ascontiguousarray(bf_eff.reshape(32, 128).T),
        "WF2": WF28,
        "WF2R": WF2R8,
        "BF2": np.ascontiguousarray(b_fc2[None, :] * 128.0),
    }

    am_zero = bool(np.all(attention_mask == 0))
    in_maps, perms = [], []
    for c in range(N_CORES):
        b, f = c >> 1, c & 1
        perm = _perm_for(f)
        perms.append(perm)
        x_ctx = np.ascontiguousarray(hidden_states[b][perm])
        # fold both the proj bias and the V bias (attention rows sum to 1,
        # so the V bias passes through softmax exactly) into the residual
        xq = np.ascontiguousarray(
            hidden_states[b][perm[OWN:]] + b_proj[None, :]
            + (BVn.astype(np.float64) @ W_proj).astype(np.float32)[None, :])
        gk = perm
        gq = perm[OWN:]
        live = (gk[:, None] <= gq[None, :]).astype(np.float32)
        # 0/1 exp-masks: [:, u*QG:(u+1)*QG] is k-block (base+u) vs q-group 0
        # pair 0: other-parity blocks (0, 1); pair 1: own blocks (8, 9).
        # The relative pattern is g-independent.
        msk = np.zeros((2, 128, 512), np.float32)
        for u, j in enumerate([0, 1]):
            msk[0, :, u * QG:(u + 1) * QG] = live[
                j * 128:(j + 1) * 128, 0:QG]
        for u, j in enumerate([8, 9]):
            msk[1, :, u * QG:(u + 1) * QG] = live[
                j * 128:(j + 1) * 128, 0:QG]
        im = {"X": bf16(x_ctx), "XQ": xq, "MSKE": bf16(msk), **shared}
        if not am_zero:
            am = attention_mask[b, 0, 0, :].astype(np.float64)
            eam = np.exp(am[perm]).astype(np.float32)
            im["EAM"] = np.ascontiguousarray(eam.reshape(SB, 128).T)
        in_maps.append(im)
    return in_maps, perms, am_zero


def kernel(hidden_states, attention_mask, ln1_g, ln1_b, W_attn, b_attn,
           W_proj, b_proj, ln2_g, ln2_b, W_fc, b_fc, W_fc2, b_fc2):
    in_maps, perms, am_zero = make_in_maps(
        hidden_states, attention_mask, ln1_g, ln1_b, W_attn, b_attn,
        W_proj, b_proj, ln2_g, ln2_b, W_fc, b_fc, W_fc2, b_fc2)
    nc = _get_nc(am_zero)
    res = run_bass_kernel_spmd(nc, in_maps, core_ids=list(range(N_CORES)))
    out = np.empty((B, S, D), dtype=np.float32)
    for c in range(N_CORES):
        b = c >> 1
        out[b][perms[c][OWN:]] = res.results[c]["OUT"]
    return out



# revision 46
# speedup vs baseline: 1.1222x; 1.0374x over previous
"""Fused GPT-2 transformer block on 8 Trainium2 NeuronCores.

Sharding: 8 cores = 4 batches x 2 causal-balanced folds. Core (b, f) owns the 8
interleaved 128-token blocks of parity f of batch b (queries), and receives all
2048 tokens of batch b as context, permuted [other-parity blocks | own blocks].
Causality is enforced by exact 0/1 mask multiplies after exp, so a single SPMD
program serves all cores. No collectives.

Layouts: LN1(x) is PE-transposed to hT [D, tok] (bf16); Q/K are produced in
head-major transposed layout (bf16), V in token-major layout with an appended
ones column (so the P@V matmul also accumulates softmax denominators).
Exp runs on the scalar engine in [128,1024] slabs straight from PSUM to bf16;
causal masking is a 0/1 elementwise multiply on the vector engine afterwards
(exp(s+m) == exp(s)*exp(m) with exp(m) in {0,1} exactly). proj/fc matmuls
contract against feature-major lhsT slices. All weights travel as bf16;
LN affines and the proj bias are folded on the host.
"""

import contextlib
import os

import numpy as np
import ml_dtypes

import concourse.bass as bass
import concourse.mybir as mybir
import concourse.tile as tile
from concourse import bacc
from concourse.bass_utils import run_bass_kernel_spmd
from concourse.masks import make_identity

F32 = mybir.dt.float32
F32R = mybir.dt.float32r
BF16 = mybir.dt.bfloat16
F8 = mybir.dt.float8e4
DR = mybir.MatmulPerfMode.DoubleRow
AF = mybir.ActivationFunctionType
ALU = mybir.AluOpType

B, S, D, H = 4, 2048, 1024, 16
HD = D // H          # 64
DFF = 4 * D          # 4096
EPS = 1e-5
MASKED_BIAS = -10000.0
N_CORES = 8

SB = S // 128        # 16 ctx blocks
OWN = S // 2         # 1024 own tokens
OB = OWN // 128      # 8 own blocks
NQG = 4              # q-groups of 256
QG = 256
HSETS = 4            # head sets
HPS = H // HSETS     # 4 heads per set


def _klist(g):
    """ctx k-block indices computed for q-group g (own blocks 2g, 2g+1)."""
    return list(range(0, 2 * g + 2)) + list(range(8, 8 + 2 * g + 2))


def build_nc(am_zero=True):
    nc = bacc.Bacc("TRN2", target_bir_lowering=False, debug=False,
                   num_devices=N_CORES)

    X = nc.dram_tensor("X", [S, D], BF16, kind="ExternalInput")
    XQ = nc.dram_tensor("XQ", [OWN, D], F32, kind="ExternalInput")
    MSKE = nc.dram_tensor("MSKE", [2, 128, 512], BF16, kind="ExternalInput")
    EAM = (None if am_zero else
           nc.dram_tensor("EAM", [128, SB], F32, kind="ExternalInput"))
    WQ = (nc.dram_tensor("WQ", [D, D], F8, kind="ExternalInput"),
          nc.dram_tensor("WQR", [D, D], F8, kind="ExternalInput"))
    WK = (nc.dram_tensor("WK", [D, D], F8, kind="ExternalInput"),
          nc.dram_tensor("WKR", [D, D], F8, kind="ExternalInput"))
    WV = (nc.dram_tensor("WV", [D, D], F8, kind="ExternalInput"),
          nc.dram_tensor("WVR", [D, D], F8, kind="ExternalInput"))
    BQ = nc.dram_tensor("BQ", [128, 8], F32, kind="ExternalInput")
    WP = (nc.dram_tensor("WP", [D, D], F8, kind="ExternalInput"),
          nc.dram_tensor("WPR", [D, D], F8, kind="ExternalInput"))
    WF = nc.dram_tensor("WF", [D, DFF], F8, kind="ExternalInput")
    WFR = nc.dram_tensor("WFR", [D, DFF], F8, kind="ExternalInput")
    BF = nc.dram_tensor("BF", [128, 32], F32, kind="ExternalInput")
    WF2 = nc.dram_tensor("WF2", [DFF, D], F8, kind="ExternalInput")
    WF2R = nc.dram_tensor("WF2R", [DFF, D], F8, kind="ExternalInput")
    BF2 = nc.dram_tensor("BF2", [1, D], F32, kind="ExternalInput")
    OUT = nc.dram_tensor("OUT", [OWN, D], F32, kind="ExternalOutput")

    with tile.TileContext(nc) as tc:
        _body(nc, tc, X, XQ, MSKE, EAM, WQ, WK, WV, BQ, WP, WF, WFR,
              BF, WF2, WF2R, BF2, OUT, am_zero)
    nc.compile()
    return nc


def _ln_stats(nc, stat, src, eps_t):
    """LN stats of src [128, D] -> (rinv [128,1], nb [128,1]) with
    nb = -mean * rinv."""
    sub = 512
    nsub = D // sub
    xs = src.rearrange("p (n s) -> p n s", s=sub)
    stats = stat.tile([128, nsub, nc.vector.BN_STATS_DIM], F32, tag="bnst")
    for j in range(nsub):
        nc.vector.bn_stats(out=stats[:, j, :], in_=xs[:, j, :])
    mv = stat.tile([128, nc.vector.BN_AGGR_DIM], F32, tag="bnag")
    nc.vector.bn_aggr(out=mv[:, :], in_=stats[:, :, :])
    nc.scalar.activation(out=mv[:, 1:2], in_=mv[:, 1:2], func=AF.Sqrt,
                         bias=eps_t[:], scale=1.0)
    rinv = stat.tile([128, 1], F32, tag="rinv")
    nc.vector.reciprocal(out=rinv[:], in_=mv[:, 1:2])
    nb = stat.tile([128, 1], F32, tag="nb")
    nc.vector.scalar_tensor_tensor(out=nb[:], in0=mv[:, 0:1], scalar=-1.0,
                                   in1=rinv[:], op0=ALU.mult, op1=ALU.mult)
    return rinv, nb


def _body(nc, tc, X, XQ, MSKE, EAM, WQ, WK, WV, BQ, WP, WF, WFR,
          BF, WF2, WF2R, BF2, OUT, am_zero=True):
    PL = int(os.environ.get("KPHASES", "9"))
    with contextlib.ExitStack() as top:
        cst = top.enter_context(tc.tile_pool(name="cst", bufs=1))
        stat = top.enter_context(tc.tile_pool(name="stat", bufs=4))

        ident = cst.tile([128, 128], F32)
        make_identity(nc, ident[:])
        ones_f = cst.tile([1, 128], F32)
        nc.vector.memset(ones_f[:], 1.0)
        # V's appended denominator column is 512 so the P@V matmul's
        # denominator absorbs the 1/512 psum scale of vS (ratio is exact)
        ones_c3 = cst.tile([128, HPS, 1], BF16)
        nc.vector.memset(ones_c3[:], 512.0)
        ones_r = cst.tile([1, 128], F32R)   # bias-row lhsT
        nc.scalar.copy(ones_r[:], ones_f[:])
        eps_t = cst.tile([128, 1], F32)
        nc.vector.memset(eps_t[:], EPS)
        nlog8_t = cst.tile([128, 1], F32)   # -ln(8): exp-shift bias
        nc.vector.memset(nlog8_t[:], -2.0794415416798357)
        ident_b = cst.tile([128, 128], BF16)
        nc.scalar.copy(ident_b[:], ident[:])

        with contextlib.ExitStack() as attn_stack:
            atp = attn_stack.enter_context(tc.tile_pool(name="atp", bufs=1))
            aT = [atp.tile([128, OWN], BF16, tag=f"aT{p}", name=f"aT{p}")
                  for p in range(8)]

            with contextlib.ExitStack() as ht_stack:
                ht = ht_stack.enter_context(tc.tile_pool(name="ht", bufs=1))
                # hT8[tg] : [128, db, 512] fp8 ~ 8*LN1(x).T plus remainder
                # hTe8 (tri-fp8 DoubleRow moving operands, db-pairs adjacent)
                hT8 = [ht.tile([128, 8, 512], F8, tag=f"hT8_{tg}",
                               name=f"hT8_{tg}") for tg in range(4)]
                hTe8 = [ht.tile([128, 8, 512], F8, tag=f"hTe8_{tg}",
                                name=f"hTe8_{tg}") for tg in range(4)]

                with contextlib.ExitStack() as hs_stack:
                    # attention-phase pools are created BEFORE the phase-1
                    # pools so phase-1 buffer teardown never aliases them
                    kvq = hs_stack.enter_context(
                        tc.tile_pool(name="kvq", bufs=2))
                    mskp = hs_stack.enter_context(
                        tc.tile_pool(name="mskp", bufs=1))
                    att = hs_stack.enter_context(
                        tc.tile_pool(name="att", bufs=5))
                    wst = hs_stack.enter_context(
                        tc.tile_pool(name="wstA", bufs=2))
                    psKV = hs_stack.enter_context(
                        tc.tile_pool(name="psKV", bufs=2, space="PSUM"))
                    psS = hs_stack.enter_context(
                        tc.tile_pool(name="psS", bufs=2, space="PSUM"))

                    # first X chunk goes to the head of the DMA queue so
                    # LN1 starts as early as possible
                    x_first = mskp.tile([128, 2, D], BF16, tag="xf",
                                        name="xf")
                    nc.sync.dma_start(
                        x_first[:],
                        X[0:256, :].rearrange("(i p) d -> p i d", p=128))

                    # 0/1 exp-masks (bf16) + per-token exp(attn-mask)
                    mskE = mskp.tile([128, 2, 512], BF16, tag="mskE",
                                     name="mskE")
                    nc.sync.dma_start(mskE[:],
                                      MSKE[:, :, :].rearrange("m p f -> p m f"))
                    eam = None
                    if not am_zero:
                        eam = mskp.tile([128, SB], F32, tag="eam", name="eam")
                        nc.sync.dma_start(eam[:], EAM[:, :])
                    bq_t = mskp.tile([128, 8], F32, tag="bq", name="bq")
                    nc.sync.dma_start(bq_t[:], BQ[:, :])

                    def tri(ps_t, wa, wr, tg, i, first, last):
                        lhA = wa[:, 2 * i:2 * i + 2, :]
                        lhR = wr[:, 2 * i:2 * i + 2, :]
                        rhA = hT8[tg][:, 2 * i:2 * i + 2, :]
                        rhE = hTe8[tg][:, 2 * i:2 * i + 2, :]
                        nc.tensor.matmul(ps_t, lhA, rhA, start=first,
                                         stop=False, perf_mode=DR)
                        nc.tensor.matmul(ps_t, lhR, rhA, start=False,
                                         stop=False, perf_mode=DR)
                        nc.tensor.matmul(ps_t, lhA, rhE, start=False,
                                         stop=last, perf_mode=DR)

                    def alloc_kvq():
                        kT = [kvq.tile([128, S], BF16, tag=f"kT{p}",
                                       name=f"kT{p}") for p in range(2)]
                        qT = [kvq.tile([128, OWN], BF16, tag=f"qT{p}",
                                       name=f"qT{p}") for p in range(2)]
                        vS = [kvq.tile([128, HPS, HD + 1], BF16,
                                       tag=f"vS{tb}", name=f"vS{tb}")
                              for tb in range(SB)]
                        return kT, qT, vS

                    def emit_wdma(hs):
                        wkq = []
                        for p in range(2):
                            fcol = hs * 256 + p * 128
                            wk_p, wq_p = [], []
                            for wt, dr in (("a", 0), ("r", 1)):
                                wk_t = wst.tile([128, 8, 128], F8,
                                                tag=f"wk{p}{wt}",
                                                name=f"wk{p}{wt}")
                                nc.sync.dma_start(
                                    wk_t[:],
                                    WK[dr][:, fcol:fcol + 128].rearrange(
                                        "(i p2) f -> p2 i f", p2=128))
                                wk_p.append(wk_t)
                                wq_t = wst.tile([128, 8, 128], F8,
                                                tag=f"wq{p}{wt}",
                                                name=f"wq{p}{wt}")
                                nc.sync.dma_start(
                                    wq_t[:],
                                    WQ[dr][:, fcol:fcol + 128].rearrange(
                                        "(i p2) f -> p2 i f", p2=128))
                                wq_p.append(wq_t)
                            wkq.append((wk_p, wq_p))
                        wv_p = []
                        for wt, dr in (("a", 0), ("r", 1)):
                            wv_t = wst.tile([128, 8, 256], F8,
                                            tag=f"wv{wt}", name=f"wv{wt}")
                            nc.sync.dma_start(
                                wv_t[:],
                                WV[dr][:, hs * 256:(hs + 1) * 256].rearrange(
                                    "(i p2) f -> p2 i f", p2=128))
                            wv_p.append(wv_t)
                        return wkq, wv_p

                    def kvq_groups(hs, tiles, weights, phase1=False):
                        """One psum-group emitter per yield; interleaved
                        into the previous head set's attention (or phase 1
                        for hs=0) so the PE has ready work while it waits
                        on exp (resp. LN1)."""
                        kT, qT, vS = tiles
                        wkq, wv_p = weights

                        def k_grp(p, tg):
                            (wk8, wkr8), _ = wkq[p]
                            ps = psKV.tile([128, 512], F32, tag="pk")
                            for i in range(4):
                                tri(ps[:], wk8, wkr8, tg, i,
                                    i == 0, i == 3)
                            dst = kT[p][:, tg * 512:(tg + 1) * 512]
                            if tg % 2 == 0:
                                nc.vector.tensor_copy(dst, ps[:])
                            else:
                                nc.scalar.copy(dst, ps[:])

                        def q_grp(p, tg):
                            _, (wq8, wqr8) = wkq[p]
                            bcol = hs * 2 + p
                            ps = psKV.tile([128, 512], F32, tag="pk")
                            for i in range(4):
                                tri(ps[:], wq8, wqr8, 2 + tg, i,
                                    i == 0, i == 3)
                            nc.vector.tensor_scalar_add(
                                out=qT[p][:, tg * 512:(tg + 1) * 512],
                                in0=ps[:],
                                scalar1=bq_t[:, bcol:bcol + 1])

                        def v_grp(tb):
                            ps = psKV.tile([128, 512], F32, tag="pk")
                            tgv, c0 = tb // 4, (tb % 4) * 128
                            for i in range(4):
                                lhA = hT8[tgv][:, 2 * i:2 * i + 2,
                                               c0:c0 + 128]
                                lhE = hTe8[tgv][:, 2 * i:2 * i + 2,
                                                c0:c0 + 128]
                                nc.tensor.matmul(
                                    ps[:, 0:256], lhA,
                                    wv_p[0][:, 2 * i:2 * i + 2, :],
                                    start=(i == 0), stop=False,
                                    perf_mode=DR)
                                nc.tensor.matmul(
                                    ps[:, 0:256], lhA,
                                    wv_p[1][:, 2 * i:2 * i + 2, :],
                                    start=False, stop=False, perf_mode=DR)
                                nc.tensor.matmul(
                                    ps[:, 0:256], lhE,
                                    wv_p[0][:, 2 * i:2 * i + 2, :],
                                    start=False, stop=(i == 3),
                                    perf_mode=DR)
                            nc.vector.tensor_copy(
                                vS[tb][:, :, 0:HD],
                                ps[:, 0:256].rearrange(
                                    "p (h d) -> p h d", d=HD))
                            nc.gpsimd.tensor_copy(
                                vS[tb][:, :, HD:HD + 1], ones_c3[:])

                        if phase1:
                            # ordered by hT8[tg] availability: after xg is
                            # transposed, its K/V (and Q once own-token
                            # groups exist) can run
                            plan = []
                            for xg in range(4):
                                chunk = [(k_grp, (0, xg)), (k_grp, (1, xg))]
                                if xg >= 2:
                                    chunk += [(q_grp, (0, xg - 2)),
                                              (q_grp, (1, xg - 2))]
                                chunk += [(v_grp, (tb,))
                                          for tb in range(4 * xg,
                                                          4 * xg + 4)]
                                plan.append(chunk)
                            for chunk in plan:
                                yield [lambda f=f, a=a: f(*a)
                                       for f, a in chunk]
                        else:
                            for p in range(2):
                                for tg in range(4):
                                    yield lambda p=p, tg=tg: k_grp(p, tg)
                                for tg in range(2):
                                    yield lambda p=p, tg=tg: q_grp(p, tg)
                            for tb in range(SB):
                                yield lambda tb=tb: v_grp(tb)

                    # attention output duals (natural fp8 scale; the
                    # subnormal tail of aT8 lands in aTe8) + bf16 partial
                    # accumulator for the proj contraction pairs of head
                    # sets 0-2, which run interleaved into head set 3
                    aT8 = atp.tile([128, 8, OWN], F8, tag="aT8",
                                   name="aT8")
                    aTe8 = atp.tile([128, 8, OWN], F8, tag="aTe8",
                                    name="aTe8")
                    pacc = atp.tile([128, 16, 512], BF16, tag="pacc",
                                    name="pacc")
                    wstp = attn_stack.enter_context(
                        tc.tile_pool(name="wstP", bufs=1))
                    wpt = []
                    for fg in range(2):
                        pair = []
                        for wt, dr in (("a", 0), ("r", 1)):
                            w_t = wstp.tile([128, 8, 512], F8,
                                            tag=f"wp{fg}{wt}",
                                            name=f"wp{fg}{wt}")
                            nc.sync.dma_start(
                                w_t[:],
                                WP[dr][:, fg * 512:
                                       (fg + 1) * 512].rearrange(
                                    "(i p2) f -> p2 i f", p2=128))
                            pair.append(w_t)
                        wpt.append(pair)

                    def proj_pair(ps_t, i, tb, fg, first, last):
                        lhA = aT8[:, 2 * i:2 * i + 2,
                                  tb * 128:(tb + 1) * 128]
                        lhE = aTe8[:, 2 * i:2 * i + 2,
                                   tb * 128:(tb + 1) * 128]
                        rhA = wpt[fg][0][:, 2 * i:2 * i + 2, :]
                        rhR = wpt[fg][1][:, 2 * i:2 * i + 2, :]
                        nc.tensor.matmul(ps_t, lhA, rhA, start=first,
                                         stop=False, perf_mode=DR)
                        nc.tensor.matmul(ps_t, lhA, rhR, start=False,
                                         stop=False, perf_mode=DR)
                        nc.tensor.matmul(ps_t, lhE, rhA, start=False,
                                         stop=last, perf_mode=DR)

                    def proj_partial_groups():
                        for tb in range(OB):
                            for fg in range(2):
                                def pgrp(tb=tb, fg=fg):
                                    ps = psKV.tile([128, 512], F32,
                                                   tag="pk")
                                    for i in range(3):
                                        proj_pair(ps[:], i, tb, fg,
                                                  i == 0, i == 2)
                                    nc.vector.tensor_copy(
                                        pacc[:, 2 * tb + fg, :], ps[:])
                                yield pgrp

                    cur_tiles = alloc_kvq()
                    cur_w = emit_wdma(0)
                    kvq0_chunks = kvq_groups(0, cur_tiles, cur_w,
                                             phase1=True)

                    # ---- Phase 1: LN1 over ctx + transpose -> hT ----
                    # 4 token-blocks transpose into one PSUM bank, so each
                    # hT[db][tg] tile is produced by a single wide copy.
                    with tc.tile_pool(name="psT", bufs=2, space="PSUM") \
                            as psT, \
                         tc.tile_pool(name="xin1", bufs=4) as xin, \
                         tc.tile_pool(name="xln", bufs=6) as xlnp:
                        for xg in range(4):
                            xts = []
                            for xh in range(2):
                                if xg == 0 and xh == 0:
                                    xts.append(x_first)
                                    continue
                                x_t = xin.tile([128, 2, D], BF16, tag="x1")
                                nc.sync.dma_start(
                                    x_t[:],
                                    X[xg * 512 + xh * 256:
                                      xg * 512 + (xh + 1) * 256,
                                      :].rearrange("(i p) d -> p i d",
                                                   p=128))
                                xts.append(x_t)
                            xls = []
                            for i in range(4):
                                xv = xts[i // 2][:, i % 2, :]
                                rinv, nb = _ln_stats(nc, stat, xv, eps_t)
                                x_ln = xlnp.tile([128, D], BF16, tag="xln")
                                nc.scalar.activation(out=x_ln[:], in_=xv,
                                                     func=AF.Identity,
                                                     bias=nb[:],
                                                     scale=rinv[:])
                                xls.append(x_ln)
                            for db in range(8):
                                pt = psT.tile([128, 512], BF16, tag="tp")
                                for i in range(4):
                                    nc.tensor.transpose(
                                        pt[:, i * 128:(i + 1) * 128],
                                        xls[i][:, db * 128:(db + 1) * 128],
                                        ident_b[:])
                                nc.scalar.activation(
                                    out=hT8[xg][:, db, :], in_=pt[:],
                                    func=AF.Copy, scale=8.0)
                                nc.vector.scalar_tensor_tensor(
                                    out=hTe8[xg][:, db, :], in0=pt[:],
                                    scalar=8.0, in1=hT8[xg][:, db, :],
                                    op0=ALU.mult, op1=ALU.subtract)
                            # head set 0's K/V/Q, one token group behind
                            # the LN1 pipeline (so the PE is not head-of-
                            # line blocked on fresh hTe8 conversions)
                            if xg >= 1:
                                for thunk in next(kvq0_chunks):
                                    thunk()
                        for chunk in kvq0_chunks:
                            for thunk in chunk:
                                thunk()

                    if PL < 2:
                        return
                    # psA reuses psT's freed banks; the region-reuse
                    # dependency (first pa write after last phase-1
                    # transpose copy) is subsumed by the data dependency
                    # attention -> K/V -> hT -> phase 1.
                    psA = hs_stack.enter_context(
                        tc.tile_pool(name="psA", bufs=2, space="PSUM"))
                    for hs in range(HSETS):
                        kT, qT, vS = cur_tiles
                        if hs + 1 < HSETS:
                            nxt_tiles = alloc_kvq()
                            nxt_w = emit_wdma(hs + 1)
                            pending = kvq_groups(hs + 1, nxt_tiles, nxt_w)
                        else:
                            nxt_tiles = nxt_w = None
                            pending = proj_partial_groups()

                        # ---- attention for this head set ----
                        # PV runs as P.T @ V: the exp block [128k, 128q] is
                        # the stationary operand and V [128k, 65] the bf16
                        # moving operand (65 rows/matmul). The ones column
                        # of V accumulates softmax denominators into the
                        # output's col 64, so normalization is a cheap
                        # per-partition scalar multiply; a PE transpose
                        # brings the normalized [q, feat] block back to
                        # feature-major aT for proj.
                        for g in range(NQG):
                            kl = _klist(g)
                            nquads = g + 1
                            for p in range(2):
                                # one bank per (g, p): accumulators for the
                                # two 64-row head-subs at cols 0:65/128:193
                                # (sub 0) and 256:321/384:449 (sub 1); the
                                # normalized output is then PE-transposed
                                # back over cols 0:128 (after the
                                # accumulators are read) so ONE [128,256]
                                # copy drains both subs into aT.
                                pab = psA.tile([128, 2 * QG], F32, tag="pab")
                                pq = [[pab[:, 0:HD + 1],
                                       pab[:, 128:128 + HD + 1]],
                                      [pab[:, 256:256 + HD + 1],
                                       pab[:, 384:384 + HD + 1]]]
                                for sub in range(2):
                                    h = 2 * p + sub
                                    for qd in range(nquads):
                                        blocks = kl[4 * qd:4 * qd + 4]
                                        pss = psS.tile([128, 1024], F32,
                                                       tag="ps")
                                        for u in range(4):
                                            kb = blocks[u]
                                            nc.tensor.matmul(
                                                pss[:, u * QG:(u + 1) * QG],
                                                kT[p][sub * 64:
                                                      (sub + 1) * 64,
                                                      kb * 128:
                                                      (kb + 1) * 128],
                                                qT[p][sub * 64:
                                                      (sub + 1) * 64,
                                                      g * QG:(g + 1) * QG],
                                                start=True, stop=True)
                                        # next head set's K/V/Q: ready PE
                                        # work while exp runs on Act
                                        grp = next(pending, None)
                                        if grp is not None:
                                            grp()
                                        wide = att.tile([128, 1024], BF16,
                                                        tag="wide",
                                                        name="wide")
                                        # psum holds 2^21 * logits (512*k x
                                        # 4096*q); the -ln(8) bias keeps exp
                                        # outputs small (softmax-invariant)
                                        nc.scalar.activation(
                                            wide[:], pss[:], AF.Exp,
                                            bias=nlog8_t[:],
                                            scale=2.0 ** -21)
                                        if qd == g // 2:
                                            sl = wide[:, (g % 2) * 512:
                                                      (g % 2) * 512 + 512]
                                            nc.vector.tensor_mul(
                                                sl, sl, mskE[:, 0, :])
                                        if qd == g:
                                            sl = wide[:, 512:1024]
                                            nc.vector.tensor_mul(
                                                sl, sl, mskE[:, 1, :])
                                        if not am_zero:
                                            for u in range(4):
                                                kb = blocks[u]
                                                sl = wide[:, u * QG:
                                                          (u + 1) * QG]
                                                nc.vector.tensor_scalar_mul(
                                                    out=sl, in0=sl,
                                                    scalar1=eam[:,
                                                                kb:kb + 1])
                                        # one accumulation group for the
                                        # whole bank: start clears the
                                        # bank-wide has_written bits, so
                                        # only the very first matmul may
                                        # carry it
                                        for u in range(4):
                                            kb = blocks[u]
                                            for qs in range(2):
                                                nc.tensor.matmul(
                                                    pq[sub][qs],
                                                    wide[:,
                                                         u * QG + qs * 128:
                                                         u * QG + qs * 128
                                                         + 128],
                                                    vS[kb][:, h, :],
                                                    start=(sub == 0
                                                           and qd == 0
                                                           and u == 0
                                                           and qs == 0),
                                                    stop=(sub == 1
                                                          and qd ==
                                                          nquads - 1
                                                          and u == 3
                                                          and qs == 1),
                                                    skip_group_check=True)
                                ap_idx = 2 * hs + p
                                anrms = []
                                for sub in range(2):
                                    for qs in range(2):
                                        rec = att.tile([128, 1], F32,
                                                       tag="rec")
                                        nc.vector.reciprocal(
                                            rec[:],
                                            pq[sub][qs][:, HD:HD + 1])
                                        anrm = att.tile([128, HD], BF16,
                                                        tag="anrm")
                                        nc.vector.tensor_scalar_mul(
                                            out=anrm[:],
                                            in0=pq[sub][qs][:, 0:HD],
                                            scalar1=rec[:])
                                        anrms.append((sub, qs, anrm))
                                for sub, qs, anrm in anrms:
                                    nc.tensor.transpose(
                                        pab[sub * 64:sub * 64 + HD,
                                            qs * 64:(qs + 1) * 64].bitcast(
                                                BF16),
                                        anrm[:], ident_b[:])
                                dst = aT[ap_idx][:, g * QG:(g + 1) * QG]
                                src = pab[:, 0:128].bitcast(BF16)
                                if p == 0:
                                    nc.vector.tensor_copy(dst, src)
                                else:
                                    nc.scalar.copy(dst, src)
                        for grp in pending:
                            grp()
                        cur_tiles, cur_w = nxt_tiles, nxt_w
                        for ab in (2 * hs, 2 * hs + 1):
                            nc.gpsimd.tensor_copy(aT8[:, ab, :],
                                                  aT[ab][:])
                            nc.gpsimd.tensor_tensor(
                                out=aTe8[:, ab, :], in0=aT[ab][:],
                                in1=aT8[:, ab, :], op=ALU.subtract)

            if PL < 4:
                return
            # ---- proj + residual -> x2 ; prefetch WF/BF/BF2 ----
            psT2 = top.enter_context(
                tc.tile_pool(name="psT2", bufs=2, space="PSUM"))
            psF = top.enter_context(
                tc.tile_pool(name="psF", bufs=2, space="PSUM"))
            x2p = top.enter_context(tc.tile_pool(name="x2p", bufs=1,
                                                 side="right"))
            wfp = top.enter_context(tc.tile_pool(name="wfp", bufs=1,
                                                 side="right"))

            x2 = [x2p.tile([128, 4, D], F32, tag=f"x2{i}", name=f"x2{i}")
                  for i in range(2)]
            for i in range(2):
                nc.sync.dma_start(
                    x2[i][:],
                    XQ[i * 512:(i + 1) * 512, :].rearrange(
                        "(i2 p) d -> p i2 d", p=128))

            wf8_t = wfp.tile([128, 8, DFF], F8, tag="wf8", name="wf8")
            wfr8_t = wfp.tile([128, 8, DFF], F8, tag="wfr8", name="wfr8")
            for db in range(8):
                nc.sync.dma_start(wf8_t[:, db, :],
                                  WF[db * 128:(db + 1) * 128, :])
                nc.sync.dma_start(wfr8_t[:, db, :],
                                  WFR[db * 128:(db + 1) * 128, :])
            bf_t = wfp.tile([128, 32], F32, tag="bf", name="bf")
            nc.sync.dma_start(bf_t[:], BF[:, :])
            bf2_t = wfp.tile([1, D], F32R, tag="bf2", name="bf2")
            nc.sync.dma_start(bf2_t[:], BF2[:, :].bitcast(F32R))

            def x2v(tb):
                return x2[tb // 4][:, tb % 4, :]

            with tc.tile_pool(name="psP", bufs=2, space="PSUM") as psP:
                for tb in range(OB):
                    for fg in range(2):
                        ps = psP.tile([128, 512], F32, tag="pp")
                        # re-inject the head-set 0-2 partials, add head
                        # set 3's contraction pair
                        nc.tensor.matmul(ps[:], ident_b[:],
                                         pacc[:, 2 * tb + fg, :],
                                         start=True, stop=False)
                        proj_pair(ps[:], 3, tb, fg, False, True)
                        dst = x2v(tb)[:, fg * 512:(fg + 1) * 512]
                        nc.vector.scalar_tensor_tensor(
                            out=dst, in0=ps[:], scalar=2.0 ** -6,
                            in1=dst, op0=ALU.mult, op1=ALU.add)

        if PL < 5:
            return
        # ---- LN2 + transpose -> h2T8 (+ fp8 residual) ; then MLP ----
        with contextlib.ExitStack() as mlp_stack:
            ht2 = mlp_stack.enter_context(tc.tile_pool(name="ht2", bufs=1))
            # fp8 dual representation of LN2(x2).T: h2T8 ~ 8*h2, h2Te8 the
            # quantization remainder (tri-fp8 DoubleRow matmul operands)
            h2T8 = [ht2.tile([128, 8, 512], F8, tag=f"h2T8_{tg}",
                             name=f"h2T8_{tg}") for tg in range(2)]
            h2Te8 = [ht2.tile([128, 8, 512], F8, tag=f"h2Te8_{tg}",
                              name=f"h2Te8_{tg}") for tg in range(2)]
            with tc.tile_pool(name="xln2", bufs=6) as xlnp:
                for tg in range(2):
                    xls = []
                    for i in range(4):
                        tb = tg * 4 + i
                        rinv, nb = _ln_stats(nc, stat, x2v(tb), eps_t)
                        x_ln = xlnp.tile([128, D], BF16, tag="xln")
                        nc.scalar.activation(out=x_ln[:], in_=x2v(tb),
                                             func=AF.Identity,
                                             bias=nb[:], scale=rinv[:])
                        xls.append(x_ln)
                    for db in range(8):
                        pt = psT2.tile([128, 512], BF16, tag="tp")
                        for i in range(4):
                            nc.tensor.transpose(
                                pt[:, i * 128:(i + 1) * 128],
                                xls[i][:, db * 128:(db + 1) * 128],
                                ident_b[:])
                        nc.scalar.activation(
                            out=h2T8[tg][:, db, :], in_=pt[:],
                            func=AF.Copy, scale=8.0)
                        nc.vector.scalar_tensor_tensor(
                            out=h2Te8[tg][:, db, :], in0=pt[:], scalar=8.0,
                            in1=h2T8[tg][:, db, :], op0=ALU.mult,
                            op1=ALU.subtract)

            if PL < 6:
                return
            with contextlib.ExitStack() as mlp2:
                gtp = mlp2.enter_context(tc.tile_pool(name="gtp", bufs=1))
                gtbp = mlp2.enter_context(tc.tile_pool(name="gtb", bufs=4))
                wst6 = mlp2.enter_context(tc.tile_pool(name="wstF6", bufs=4))
                outp = mlp2.enter_context(tc.tile_pool(name="outp", bufs=3))
                psO = None

                def fc2_tri(pso_t, jj, pr, tb, gt8, ge8, w2a, w2r):
                    jp = jj * 4 + pr
                    j0 = 2 * jp
                    lhA = gt8[:, j0:j0 + 2, tb * 128:(tb + 1) * 128]
                    lhE = ge8[:, j0:j0 + 2, tb * 128:(tb + 1) * 128]
                    rhA = w2a[:, 2 * pr:2 * pr + 2, :]
                    rhR = w2r[:, 2 * pr:2 * pr + 2, :]
                    nc.tensor.matmul(pso_t[:], lhA, rhA, start=(jp == 0),
                                     stop=False, perf_mode=DR)
                    nc.tensor.matmul(pso_t[:], lhA, rhR, start=False,
                                     stop=False, perf_mode=DR)
                    nc.tensor.matmul(pso_t[:], lhE, rhA, start=False,
                                     stop=False, perf_mode=DR)

                for tg in range(2):
                    # gelu output in dual fp8: gt8 = fp8(gelu), ge8 the
                    # bf16-vs-fp8 remainder (tri-fp8 fc2 stationaries)
                    gt8 = gtp.tile([128, 32, 512], F8, tag="gt8",
                                   name="gt8")
                    ge8 = gtp.tile([128, 32, 512], F8, tag="ge8",
                                   name="ge8")
                    for j in range(32):
                        ps = psF.tile([128, 512], F32, tag="pf")
                        for i in range(4):
                            wA = wf8_t[:, 2 * i:2 * i + 2,
                                       j * 128:(j + 1) * 128]
                            wC = wfr8_t[:, 2 * i:2 * i + 2,
                                        j * 128:(j + 1) * 128]
                            hA = h2T8[tg][:, 2 * i:2 * i + 2, :]
                            hB = h2Te8[tg][:, 2 * i:2 * i + 2, :]
                            nc.tensor.matmul(ps[:], wA, hA, start=(i == 0),
                                             stop=False, perf_mode=DR)
                            nc.tensor.matmul(ps[:], wC, hA, start=False,
                                             stop=False, perf_mode=DR)
                            nc.tensor.matmul(ps[:], wA, hB, start=False,
                                             stop=(i == 3), perf_mode=DR)
                        gt_b = gtbp.tile([128, 512], BF16, tag="gtb")
                        nc.scalar.activation(gt_b[:], ps[:],
                                             AF.Gelu_apprx_tanh,
                                             bias=bf_t[:, j:j + 1],
                                             scale=1.0 / 512)
                        nc.vector.tensor_copy(gt8[:, j, :], gt_b[:])
                        nc.vector.tensor_tensor(
                            out=ge8[:, j, :], in0=gt_b[:], in1=gt8[:, j, :],
                            op=ALU.subtract)
                    if psO is None:
                        psO = mlp2.enter_context(
                            tc.tile_pool(name="psO", bufs=1, space="PSUM"))
                    for fg in range(2):
                        last = (tg == 1 and fg == 1)
                        pso = [psO.tile([128, 512], F32, tag=f"po{tb}",
                                        name=f"po{tb}") for tb in range(4)]
                        w2s = []
                        for jj in range(4):
                            w2a = wst6.tile([128, 8, 512], F8, tag="wf2a",
                                            name="wf2a")
                            nc.sync.dma_start(
                                w2a[:],
                                WF2[jj * 1024:(jj + 1) * 1024,
                                    fg * 512:(fg + 1) * 512].rearrange(
                                        "(i p2) f -> p2 i f", p2=128))
                            w2r = wst6.tile([128, 8, 512], F8, tag="wf2r",
                                            name="wf2r")
                            nc.sync.dma_start(
                                w2r[:],
                                WF2R[jj * 1024:(jj + 1) * 1024,
                                     fg * 512:(fg + 1) * 512].rearrange(
                                         "(i p2) f -> p2 i f", p2=128))
                            w2s.append((w2a, w2r))
                            if last:
                                continue
                            for pr in range(4):
                                for tb in range(4):
                                    fc2_tri(pso[tb], jj, pr, tb, gt8, ge8,
                                            w2a, w2r)
                        for tb in range(4):
                            if last:
                                # tb-major on the final pass: each output
                                # block drains (bias/add/store) while the
                                # next accumulates, hiding the tail chain
                                for jj in range(4):
                                    for pr in range(4):
                                        fc2_tri(pso[tb], jj, pr, tb, gt8,
                                                ge8, w2s[jj][0], w2s[jj][1])
                            nc.tensor.matmul(
                                pso[tb][:], ones_r[:],
                                bf2_t[0:1, fg * 512:(fg + 1) * 512],
                                start=False, stop=True)
                            gtb = tg * 4 + tb
                            o_t = outp.tile([128, 512], F32, tag="ot")
                            nc.vector.scalar_tensor_tensor(
                                out=o_t[:], in0=pso[tb][:],
                                scalar=1.0 / 128,
                                in1=x2v(gtb)[:, fg * 512:(fg + 1) * 512],
                                op0=ALU.mult, op1=ALU.add)
                            nc.scalar.dma_start(
                                OUT[gtb * 128:(gtb + 1) * 128,
                                    fg * 512:(fg + 1) * 512], o_t[:])


_NC_CACHE = {}


def _get_nc(am_zero=True):
    key = f"nc{int(am_zero)}"
    if key not in _NC_CACHE:
        _NC_CACHE[key] = build_nc(am_zero)
    return _NC_CACHE[key]


def _perm_for(f):
    other = [2 * j + (1 - f) for j in range(8)]
    own = [2 * j + f for j in range(8)]
    blocks = other + own
    return np.concatenate([np.arange(b * 128, (b + 1) * 128) for b in blocks])


def make_in_maps(hidden_states, attention_mask, ln1_g, ln1_b, W_attn, b_attn,
                 W_proj, b_proj, ln2_g, ln2_b, W_fc, b_fc, W_fc2, b_fc2):
    f32 = lambda a: np.asarray(a, dtype=np.float32)
    bf16 = lambda a: np.ascontiguousarray(a.astype(ml_dtypes.bfloat16))
    hidden_states = f32(hidden_states)
    attention_mask = f32(attention_mask)
    ln1_g, ln1_b = f32(ln1_g), f32(ln1_b)
    ln2_g, ln2_b = f32(ln2_g), f32(ln2_b)
    W_attn, b_attn = f32(W_attn), f32(b_attn)
    W_proj, b_proj = f32(W_proj), f32(b_proj)
    W_fc, b_fc = f32(W_fc), f32(b_fc)
    W_fc2, b_fc2 = f32(W_fc2), f32(b_fc2)

    # Fold LN affines into the consuming matmuls (exact algebra, fp64 on host).
    Wa_eff = (ln1_g.astype(np.float64)[:, None] * W_attn).astype(np.float32)
    ba_eff = (b_attn.astype(np.float64)
              + ln1_b.astype(np.float64) @ W_attn).astype(np.float32)
    scale = 1.0 / np.sqrt(np.float32(HD))
    WQn = (Wa_eff[:, 0:D] * scale).astype(np.float32)
    BQn = (ba_eff[0:D] * scale).astype(np.float32)
    WKn, BKn = Wa_eff[:, D:2 * D].copy(), ba_eff[D:2 * D].copy()
    WVn, BVn = Wa_eff[:, 2 * D:3 * D].copy(), ba_eff[2 * D:3 * D].copy()
    Wf_eff = (ln2_g.astype(np.float64)[:, None] * W_fc).astype(np.float32)
    bf_eff = (b_fc.astype(np.float64)
              + ln2_b.astype(np.float64) @ W_fc).astype(np.float32)

    def fp8_pair(w, s):
        """w*s as fp8 plus fp8 remainder (tri-fp8 stationaries)."""
        ws = (w.astype(np.float64) * s).astype(np.float32)
        w8 = ws.astype(ml_dtypes.float8_e4m3)
        wr8 = (ws - w8.astype(np.float32)).astype(ml_dtypes.float8_e4m3)
        assert np.isfinite(w8.astype(np.float32)).all()
        return np.ascontiguousarray(w8), np.ascontiguousarray(wr8)

    WF8, WFR8 = fp8_pair(Wf_eff, 64.0)       # fc1 psum = 8*64 * fc1
    WF28, WF2R8 = fp8_pair(W_fc2, 128.0)     # fc2 psum = 128 * fc2
    WQ8, WQR8 = fp8_pair(WQn, 512.0)         # q psum = 8*512 * q_eff
    WK8, WKR8 = fp8_pair(WKn, 64.0)          # k psum = 8*64 * k
    WV8, WVR8 = fp8_pair(WVn, 64.0)          # v psum = 8*64 * v
    WP8, WPR8 = fp8_pair(W_proj, 64.0)       # proj psum = 8*64 * proj

    shared = {
        "WQ": WQ8, "WQR": WQR8,
        "WK": WK8, "WKR": WKR8,
        "WV": WV8, "WVR": WVR8,
        # q bias at the q-psum scale (folded during psum->qT copy); the k
        # bias is dropped exactly (softmax is invariant to per-query
        # constants), the v bias is folded into the proj residual below
        "BQ": np.ascontiguousarray(BQn.reshape(8, 128).T * 4096.0),
        "WP": WP8, "WPR": WPR8,
        "WF": WF8,
        "WFR": WFR8,
        "BF": np.ascontiguousarray(bf_eff.reshape(32, 128).T),
        "WF2": WF28,
        "WF2R": WF2R8,
        "BF2": np.ascontiguousarray(b_fc2[None, :] * 128.0),
    }

    am_zero = bool(np.all(attention_mask == 0))
    in_maps, perms = [], []
    for c in range(N_CORES):
        b, f = c >> 1, c & 1
        perm = _perm_for(f)
        perms.append(perm)
        x_ctx = np.ascontiguousarray(hidden_states[b][perm])
        # fold both the proj bias and the V bias (attention rows sum to 1,
        # so the V bias passes through softmax exactly) into the residual
        xq = np.ascontiguousarray(
            hidden_states[b][perm[OWN:]] + b_proj[None, :]
            + (BVn.astype(np.float64) @ W_proj).astype(np.float32)[None, :])
        gk = perm
        gq = perm[OWN:]
        live = (gk[:, None] <= gq[None, :]).astype(np.float32)
        # 0/1 exp-masks: [:, u*QG:(u+1)*QG] is k-block (base+u) vs q-group 0
        # pair 0: other-parity blocks (0, 1); pair 1: own blocks (8, 9).
        # The relative pattern is g-independent.
        msk = np.zeros((2, 128, 512), np.float32)
        for u, j in enumerate([0, 1]):
            msk[0, :, u * QG:(u + 1) * QG] = live[
                j * 128:(j + 1) * 128, 0:QG]
        for u, j in enumerate([8, 9]):
            msk[1, :, u * QG:(u + 1) * QG] = live[
                j * 128:(j + 1) * 128, 0:QG]
        im = {"X": bf16(x_ctx), "XQ": xq, "MSKE": bf16(msk), **shared}
        if not am_zero:
            am = attention_mask[b, 0, 0, :].astype(np.float64)
            eam = np.exp(am[perm]).astype(np.float32)
            im["EAM"] = np.ascontiguousarray(eam.reshape(SB, 128).T)
        in_maps.append(im)
    return in_maps, perms, am_zero


def kernel(hidden_states, attention_mask, ln1_g, ln1_b, W_attn, b_attn,
           W_proj, b_proj, ln2_g, ln2_b, W_fc, b_fc, W_fc2, b_fc2):
    in_maps, perms, am_zero = make_in_maps(
        hidden_states, attention_mask, ln1_g, ln1_b, W_attn, b_attn,
        W_proj, b_proj, ln2_g, ln2_b, W_fc, b_fc, W_fc2, b_fc2)
    nc = _get_nc(am_zero)
    res = run_bass_kernel_spmd(nc, in_maps, core_ids=list(range(N_CORES)))
    out = np.empty((B, S, D), dtype=np.float32)
    for c in range(N_CORES):
        b = c >> 1
        out[b][perms[c][OWN:]] = res.results[c]["OUT"]
    return out

